# revision 1
# baseline (speedup 1.0000x reference)
"""Distributed ARMAConv kernel for 8 TRN2 NeuronCores (Bass/Tile).

Reference computation (N=16384 nodes, F=64 in-feats, C=32 channels,
K=2 stacks, T=2 iterations):
    for each stack k:  xbar = x
        for i in 0..1: xbar = relu(fltr @ (xbar @ w1) + x @ w2 + b)
    out = mean over stacks                                  -> [N, 32]

Strategy:
  - Row-shard fltr across 8 cores; core m holds fltr[rows_m, :] stored
    TRANSPOSED (contraction-major, two contiguous half-arrays) so every
    TensorE tile is a large contiguous DMA read.
  - fltr is stored at rest in DRAM as FP8 E3M4, pre-scaled by 2^8 on
    the host (the 2^-8 descale is folded into w1, exactly).  This cuts
    the dominant HBM stream 4x vs f32: 32 MiB per core per pass.  The
    PE consumes fp8 at bf16 speed (no DoubleRow - E4M3 would lose too
    much precision), so the kernel is TensorE-bound at ~110 us/pass.
  - Fuse the two independent ARMA stacks: Y = [xbar_k0 @ w1_k0 |
    xbar_k1 @ w1_k1] is [N, 64], so fltr streams only once per
    iteration.
  - All big matmuls run transposed (out^T = Y^T @ fltr_m^T) so fltr is
    the 512-wide moving operand (128 elem/cycle); Y tiles are the
    stationary operand (weight loads hide under the previous matmul).
  - Iteration 0 needs no communication (x is replicated).  Pass 1 runs
    in two output-row halves (full-width fltr^T streams, 1 KiB DMA
    lines - narrower strips choke the HWDGE descriptor ring); each half
    feeds TWO 512-row PSUM accumulators and fires TWO small (64 KiB)
    Y1 all-gathers, so pass 2 can consume gather chunks as they land.
    A dummy warm-up collective at t=0 absorbs the one-time rendezvous
    barrier + Mesh warm-up (~60us) that would otherwise delay gather 0.
  - Pass 2 consumes the gathered chunks contraction-major (chunk 0..2
    feed all four output accumulators, chunk 3 is processed per output
    half so the first half's epilogue hides under the second half's
    stream); chunk 3 is not needed until ~85us after pass 2 starts,
    tolerating inter-core start skew.
  - Big fltr DMAs ride the sync-engine HWDGE ring; all small/latency
    DMAs ride the scalar-engine ring so they never queue behind a
    1 MiB fltr read; collectives keep the gpsimd queue.
  - relu positive homogeneity folds the final stack-mean 0.5 scale into
    the pass-2 activation; the host only shards/quantizes inputs and
    concatenates/transposes the [32, 2048] per-core outputs.
"""

import numpy as np
import ml_dtypes

import concourse.mybir as mybir
import concourse.tile as tile
from concourse import bacc
from concourse.bass_utils import run_bass_kernel_spmd

N = 16384            # nodes
F = 64               # input features
C = 32               # channels per stack
C2 = 2 * C           # fused channels (2 stacks)
NCORES = 8
R = N // NCORES      # fltr rows per core (2048)
P = 128              # partitions
NKT = N // P         # K tiles per full pass (128)
HW_ = R // 2         # 1024 output rows per half-array
CW = 512             # output rows per pass-1 chunk / PSUM accumulator
NCH = R // CW        # 4 pass-1 chunks (each with its own all-gather)
KB1 = 4              # K tiles per pass-1 fltr DMA (512 KiB fp8 reads;
                     # 512-row tiles match pass-2's contraction blocks)
FSCALE = 256.0       # power-of-2 fp8 pre-scale (folded into w1)

F32 = mybir.dt.float32
F32R = mybir.dt.float32r
BF16 = mybir.dt.bfloat16
F8 = mybir.dt.float8e3

_CACHE = {}


def _build():
    nc = bacc.Bacc(
        trn_type="TRN2", target_bir_lowering=False, debug=False,
        num_devices=NCORES,
    )
    fltrT0_e = nc.dram_tensor("fltrt0", [N, HW_], F8, kind="ExternalInput")
    fltrT1_e = nc.dram_tensor("fltrt1", [N, HW_], F8, kind="ExternalInput")
    xT_e = nc.dram_tensor("xt", [F, N], BF16, kind="ExternalInput")
    xtm_e = nc.dram_tensor("xtm", [F, R], F32, kind="ExternalInput")
    w1i0_e = nc.dram_tensor("w1i0", [F, C2], BF16, kind="ExternalInput")
    w1i1_e = nc.dram_tensor("w1i1", [C2, C2], BF16, kind="ExternalInput")
    w2i0_e = nc.dram_tensor("w2i0", [F, C2], F32, kind="ExternalInput")
    w2i1_e = nc.dram_tensor("w2i1", [F, C2], F32, kind="ExternalInput")
    bi0_e = nc.dram_tensor("bi0", [C2, 1], F32, kind="ExternalInput")
    bi1h_e = nc.dram_tensor("bi1h", [C2, 1], F32, kind="ExternalInput")
    out_e = nc.dram_tensor("out", [C, R], F32, kind="ExternalOutput")

    RG = [list(range(NCORES))]
    fltr_halves = [fltrT0_e, fltrT1_e]

    with tile.TileContext(nc) as tc:
        with (
            tc.tile_pool(name="wpool", bufs=1) as wpool,
            tc.tile_pool(name="kpool", bufs=1) as kpool,
            tc.tile_pool(name="y0pool", bufs=1) as y0pool,
            tc.tile_pool(name="ygpool", bufs=1) as ygpool,
            tc.tile_pool(name="fpool", bufs=8) as fpool,
            tc.tile_pool(name="xbpool", bufs=2) as xbpool,
            tc.tile_pool(name="ylpool", bufs=2) as ylpool,
            tc.tile_pool(name="opool", bufs=1) as opool,
            tc.tile_pool(name="pacc", bufs=4, space="PSUM") as pacc,
            tc.tile_pool(name="psmall", bufs=2, space="PSUM") as psmall,
            tc.tile_pool(name="dram", bufs=8, space="DRAM") as dram,
        ):
            # w1i0 and the xT quarters first: they gate the first Y0 matmul
            # and thus the whole pass-1 PE start.  Four independent tiles
            # (no write-after-write chain) ride the sync ring AHEAD of the
            # fltr stream - the scalar ring's per-DMA fixed costs would
            # deliver them too slowly.
            w1i0 = wpool.tile([F, C2], BF16)
            nc.scalar.dma_start(w1i0[:], w1i0_e[:])
            xfs = []
            for g in range(4):
                xf = wpool.tile([F, N // 4], BF16, name=f"xf{g}")
                nc.sync.dma_start(xf[:],
                                  xT_e[:, g * (N // 4):(g + 1) * (N // 4)])
                xfs.append(xf)

            # dummy warm-up collective: anchors the one-time rendezvous
            # barrier at t~20us while all cores are still in startup (the
            # barrier attaches to each core's FIRST collective; without
            # this it attaches to gather-0 mid-pass and costs ~80us).
            # Collectives cannot read IO tensors: bounce w1i0 via DRAM.
            gwin = dram.tile([F, C2], BF16, name="gwin", tag="gwin")
            nc.scalar.dma_start(gwin[:], w1i0[:])
            gwout = dram.tile([NCORES * F, C2], BF16, name="gwout",
                              tag="gwout", addr_space="Shared")
            nc.gpsimd.collective_compute(
                "AllGather", mybir.AluOpType.bypass,
                replica_groups=RG,
                ins=[gwin[:].opt()], outs=[gwout[:].opt()],
            )

            # remaining resident small tensors
            w1i1 = wpool.tile([C2, C2], BF16)  # block-diag [w1_k0i1, w1_k1i1]
            nc.scalar.dma_start(w1i1[:], w1i1_e[:])
            w2i0 = wpool.tile([F, C2], F32R)
            nc.scalar.dma_start(w2i0[:], w2i0_e[:].bitcast(F32R))
            bi0 = wpool.tile([C2, 1], F32)
            nc.scalar.dma_start(bi0[:], bi0_e[:])
            xm = wpool.tile([F, R], F32R)
            nc.scalar.dma_start(xm[:], xtm_e[:].bitcast(F32R))
            w2i1 = wpool.tile([F, C2], F32R)
            nc.scalar.dma_start(w2i1[:], w2i1_e[:].bitcast(F32R))
            bi1h = wpool.tile([C2, 1], F32)
            nc.scalar.dma_start(bi1h[:], bi1h_e[:])

            y0 = y0pool.tile([P, NKT, C2], BF16, tag="y0")  # node-major Y0

            # ---- Y0 = x @ [w1_k0i0 | w1_k1i0], node-major, cast to bf16 ----
            for g in range(16):  # 16 groups of 8 kt
                xf = xfs[g // 4]
                off = (g % 4) * 1024
                ps0 = psmall.tile([P, 8, C2], F32, name="ps0", tag="ps0")
                for i in range(8):
                    nc.tensor.matmul(
                        ps0[:, i, :],
                        xf[:, off + i * P:off + (i + 1) * P],
                        w1i0[:],
                        start=True, stop=True,
                    )
                nc.vector.tensor_copy(y0[:, g * 8:(g + 1) * 8, :], ps0[:])

            # pass-2 gather-half tiles: yg tile h holds gather half h; row
            # b*128+p of gout_h is node (b//8)*2048 + h*1024 + (b%8)*128 + p.
            yg = [ygpool.tile([P, NCORES * 8, C2], BF16, name=f"yg{h}",
                              tag=f"yg{h}") for h in range(2)]
            yg_issued = [False] * 2
            gouts = []

            def issue_yg(c):
                # deferred issue: by the time it is queued the gather is
                # (normally) complete, so the scalar ring never head-blocks
                if not yg_issued[c]:
                    nc.scalar.dma_start(
                        yg[c][:],
                        gouts[c][:].rearrange("(b p) ch -> p b ch", p=P),
                    )
                    yg_issued[c] = True

            def issue_yg0():
                issue_yg(0)

            # ---- pass 1: two output-row halves (full-width 1 KiB DMA
            # ---- lines); each half fills two 512-row accumulators and
            # ---- fires two small all-gathers back-to-back
            kept = {}
            for half in range(2):
                p1 = []
                for rc2 in range(2):
                    ck = half * 2 + rc2
                    acc = pacc.tile([C2, CW], F32, name=f"p1_{ck}",
                                    tag="acc")
                    nc.tensor.matmul(
                        acc[:],
                        w2i0[:],
                        xm[:, ck * CW:(ck + 1) * CW],
                        start=True, stop=False,
                    )
                    p1.append(acc)
                for ktb in range(NKT // KB1):
                    if ktb < 16 and ktb % 4 < 2:
                        # contraction rows [j*2048, +1024) for j<4: the
                        # blocks pass-2 phase A consumes FIRST - pin them
                        # in SBUF so phase A's opening runs DMA-free while
                        # the last gather's data phase is still in flight
                        ft = kpool.tile([P, KB1, HW_], F8, name="ftk",
                                        tag="ftk", bufs=16)
                        kept[(half, ktb // 4, ktb % 4)] = ft
                    else:
                        ft = fpool.tile([P, KB1, HW_], F8, name="ft",
                                        tag="ft")
                    nc.sync.dma_start(
                        ft[:],
                        fltr_halves[half][ktb * KB1 * P:(ktb + 1) * KB1 * P,
                                          :]
                        .rearrange("(b p) c -> p b c", p=P),
                    )
                    for b in range(KB1):
                        kt = ktb * KB1 + b
                        for rc2 in range(2):
                            nc.tensor.matmul(
                                p1[rc2][:],
                                y0[:, kt, :],
                                ft[:, b, rc2 * CW:(rc2 + 1) * CW],
                                start=False, stop=(kt == NKT - 1),
                            )

                # epilogue: relu -> Y1 half (bf16) -> one small all-gather
                y1h = ylpool.tile([P, 8, C2], BF16, name="y1h")
                for rc2 in range(2):
                    xb1 = xbpool.tile([C2, CW], BF16, name="xb1")
                    nc.scalar.activation(
                        xb1[:], p1[rc2][:],
                        mybir.ActivationFunctionType.Relu,
                        bias=bi0[:], scale=1.0,
                    )
                    for t in range(4):  # node-subtiles of 128 in the chunk
                        psy = psmall.tile([P, C2], F32, name="psy",
                                          tag="psy")
                        nc.tensor.matmul(
                            psy[:],
                            xb1[:, t * P:(t + 1) * P],
                            w1i1[:],
                            start=True, stop=True,
                        )
                        nc.vector.tensor_copy(y1h[:, rc2 * 4 + t, :],
                                              psy[:])
                gin = dram.tile([HW_, C2], BF16, name="gin", tag="gin",
                                bufs=2)
                nc.scalar.dma_start(
                    gin[:].rearrange("(t p) ch -> p t ch", p=P),
                    y1h[:],
                )
                gout = dram.tile(
                    [NCORES * HW_, C2], BF16, name="gout", tag="gout",
                    addr_space="Shared", bufs=2,
                )
                nc.gpsimd.collective_compute(
                    "AllGather", mybir.AluOpType.bypass,
                    replica_groups=RG,
                    ins=[gin[:].opt()], outs=[gout[:].opt()],
                )
                gouts.append(gout)
            # gather-0 finished well before pass-1 ends; load its yg tile
            # now so pass-2's first matmuls start without a long wait
            issue_yg(0)

            outT = opool.tile([C, R], F32)

            # ---- pass 2: contraction is gathered Y1, consumed chunk-major
            p2 = []
            for rc in range(NCH):
                acc = pacc.tile([C2, CW], F32, name=f"p2_{rc}", tag="acc")
                nc.tensor.matmul(
                    acc[:],
                    w2i1[:],
                    xm[:, rc * CW:(rc + 1) * CW],
                    start=True, stop=False,
                )
                p2.append(acc)

            def p2_block(h, j, hx, rcs, stop):
                # contraction rows [j*2048 + h*1024, +1024) of half-array hx
                for b2 in range(2):
                    if h == 0 and j < 4:
                        ft = kept[(hx, j, b2)]  # pinned in SBUF since pass 1
                    else:
                        ft = fpool.tile([P, 4, HW_], F8, name="ft2",
                                        tag="ft")
                        nc.sync.dma_start(
                            ft[:],
                            fltr_halves[hx][j * R + h * HW_ + b2 * CW:
                                            j * R + h * HW_ + (b2 + 1) * CW,
                                            :]
                            .rearrange("(b p) c -> p b c", p=P),
                        )
                    for b in range(4):
                        for rc in rcs:
                            nc.tensor.matmul(
                                p2[rc][:],
                                yg[h][:, j * 8 + b2 * 4 + b, :],
                                ft[:, b, (rc % 2) * CW:(rc % 2 + 1) * CW],
                                start=False,
                                stop=stop and b2 == 1 and b == 3
                                and rc == rcs[-1],
                            )

            # phase A: gather half 0 feeds all four output accumulators;
            # j<4 runs from pinned tiles while gather 1's data phase is
            # still in flight
            issue_yg(1)
            for j in range(NCORES):
                p2_block(0, j, 0, [0, 1], False)
                p2_block(0, j, 1, [2, 3], False)

            def p2_epilogue(rc):
                xb2 = xbpool.tile([C2, CW], F32, name="xb2")
                nc.scalar.activation(
                    xb2[:], p2[rc][:], mybir.ActivationFunctionType.Relu,
                    bias=bi1h[:], scale=0.5,
                )
                # partition-shift stack-1 half to base 0 (DMA), then add
                xs = xbpool.tile([C, CW], F32, name="xs")
                nc.scalar.dma_start(xs[:], xb2[C:C2, :])
                nc.vector.tensor_add(
                    outT[:, rc * CW:(rc + 1) * CW],
                    xb2[0:C, :], xs[:],
                )

            # phase B: gather half 1 per output half; the first half's
            # epilogue hides under the second half's matmul stream
            for j in range(NCORES):
                p2_block(1, j, 0, [0, 1], j == NCORES - 1)
            for rc in (0, 1):
                p2_epilogue(rc)
            nc.scalar.dma_start(out_e[:, 0:HW_], outT[:, 0:HW_])
            for j in range(NCORES):
                p2_block(1, j, 1, [2, 3], j == NCORES - 1)
            for rc in (2, 3):
                p2_epilogue(rc)
            nc.scalar.dma_start(out_e[:, HW_:R], outT[:, HW_:R])

    nc.compile()
    return nc


def kernel(**inputs):
    x = np.ascontiguousarray(np.asarray(inputs["x"], dtype=np.float32))
    fltr = np.ascontiguousarray(np.asarray(inputs["fltr"], dtype=np.float32))

    def cat(a, b, axis=1):
        return np.ascontiguousarray(
            np.concatenate(
                [np.asarray(a, np.float32), np.asarray(b, np.float32)],
                axis=axis,
            )
        )

    f8 = ml_dtypes.float8_e3m4
    bf = ml_dtypes.bfloat16
    w1i0 = np.ascontiguousarray(
        (cat(inputs["k0i0_w1"], inputs["k1i0_w1"]) / FSCALE).astype(bf))
    w1i1f = np.zeros((C2, C2), dtype=np.float32)
    w1i1f[0:C, 0:C] = np.asarray(inputs["k0i1_w1"], np.float32)
    w1i1f[C:C2, C:C2] = np.asarray(inputs["k1i1_w1"], np.float32)
    w1i1 = np.ascontiguousarray((w1i1f / FSCALE).astype(bf))
    w2i0 = cat(inputs["k0i0_w2"], inputs["k1i0_w2"])
    w2i1 = cat(inputs["k0i1_w2"], inputs["k1i1_w2"])
    bi0 = cat(inputs["k0i0_b"], inputs["k1i0_b"], axis=0)[:, None]
    bi1h = 0.5 * cat(inputs["k0i1_b"], inputs["k1i1_b"], axis=0)[:, None]
    bi1h = np.ascontiguousarray(bi1h)
    xT = np.ascontiguousarray(x.T.astype(bf))
    # fp8 E3M4 fltr at rest: transpose per core, scale by 2^8 (descale is
    # folded into w1i0/w1i1 above; values land in [-10.9, 10.9] < 15.5 max)
    fltrs = (fltr * np.float32(FSCALE)).astype(f8)

    if "nc" not in _CACHE:
        _CACHE["nc"] = _build()
    nc = _CACHE["nc"]

    in_maps = []
    for m in range(NCORES):
        rows = slice(m * R, (m + 1) * R)
        in_maps.append({
            "fltrt0": np.ascontiguousarray(fltrs[m * R:m * R + HW_, :].T),
            "fltrt1": np.ascontiguousarray(fltrs[m * R + HW_:(m + 1) * R, :].T),
            "xt": xT,
            "xtm": np.ascontiguousarray(x[rows, :].T),
            "w1i0": w1i0, "w1i1": w1i1, "w2i0": w2i0, "w2i1": w2i1,
            "bi0": bi0, "bi1h": bi1h,
        })

    import os
    import time
    trace = os.environ.get("ARMA_TRACE") == "1"
    last_exc = None
    for attempt in range(3):
        try:
            res = run_bass_kernel_spmd(
                nc, in_maps, core_ids=list(range(NCORES)), trace=trace,
            )
            break
        except Exception as e:  # transient NRT device errors: retry
            last_exc = e
            time.sleep(5.0)
    else:
        raise last_exc
    _CACHE["last_results"] = res
    out = np.concatenate(
        [np.asarray(res.results[m]["out"]).T for m in range(NCORES)], axis=0
    )
    return out



# revision 7
# speedup vs baseline: 1.1257x; 1.1257x over previous
"""Distributed ARMAConv kernel for 8 TRN2 NeuronCores (Bass/Tile).

Reference computation (N=16384 nodes, F=64 in-feats, C=32 channels,
K=2 stacks, T=2 iterations):
    for each stack k:  xbar = x
        for i in 0..1: xbar = relu(fltr @ (xbar @ w1) + x @ w2 + b)
    out = mean over stacks                                  -> [N, 32]

Strategy (v2 - 2x column-tiled PE):
  - Row-shard fltr across 8 cores; core m holds fltr[rows_m, :] stored
    TRANSPOSED as two contiguous half-arrays (1 KiB DMA lines), fp8
    E3M4 at rest, pre-scaled by 2^8 (descale folded into w1).
  - Fuse the two ARMA stacks: Y = [xbar@w1_k0 | xbar@w1_k1] is [N,64],
    so fltr streams once per iteration.
  - The stationary operand (Y tile [128,64]) only fills half the
    128-wide PE array.  ALL matmuls run 2x column-tiled (tile_size
    (128,64)): tile (0,0) -> PSUM partitions 0-63, tile (0,64) ->
    64-127, each with its own moving fltr stream.  Measured 519 ns per
    kt-tile (2 concurrent 512-wide fp8 streams) vs 1034 serial -> PE
    ~67us per pass instead of ~110, making the kernel DMA-bound.
  - Every matmul in the kernel keeps tile_size (128,64) (no mode
    switches): K=64 matmuls (Y0 = x@w1, Y1 = relu@w1i1, w2-terms,
    final stack-mean) are zero-padded to K=128, with zeros placed in
    the operand that multiplies the junk rows.  The final stack-mean
    (out = 0.5*(relu_lo + relu_hi)) runs on the PE via a 0/1
    selection stationary instead of partition-shift DMA + DVE add.
  - The x@w2 bias term is accumulated LAST (stop) instead of first,
    so xm's DMA is off the critical path; the first fltr matmul
    carries start=True.
  - SBUF pinning: the pass-1 fltr blocks that pass-2 phase A (gather
    half 0, low core-blocks) consumes are kept resident (JPIN blocks
    per half-array stream); phase-A matmuls for those blocks are
    interleaved into pass-1 half-1's DMA-bound stream, filling PE
    idle slots.  Pass 2 re-reads only the rest.
  - Big fltr DMAs ride the sync-engine HWDGE ring; small/latency DMAs
    ride the scalar-engine ring; collectives keep the gpsimd queue.
    A dummy warm-up collective at t=0 absorbs the one-time rendezvous
    barrier (~60us) that would otherwise delay gather 0.
"""

import numpy as np
import ml_dtypes

import concourse.mybir as mybir
import concourse.tile as tile
from concourse import bacc
from concourse.bass_utils import run_bass_kernel_spmd

N = 16384            # nodes
F = 64               # input features
C = 32               # channels per stack
C2 = 2 * C           # fused channels (2 stacks)
NCORES = 8
R = N // NCORES      # fltr rows per core (2048)
P = 128              # partitions
NKT = N // P         # kt tiles per full pass (128)
HW_ = R // 2         # 1024 output rows per half-array
CW = 512             # output rows per chunk / PSUM accumulator slice
KB1 = 4              # kt tiles per fltr DMA block (512 KiB)
NBLK = NKT // KB1    # 32 blocks per half-array
JPIN = 5             # pin blocks {4j,4j+1: j<JPIN} of both half-arrays
FSCALE = 256.0       # power-of-2 fp8 pre-scale (folded into w1)

F32 = mybir.dt.float32
F32R = mybir.dt.float32r
BF16 = mybir.dt.bfloat16
F8 = mybir.dt.float8e3

_CACHE = {}


def _build():
    nc = bacc.Bacc(
        trn_type="TRN2", target_bir_lowering=False, debug=False,
        num_devices=NCORES,
    )
    fltrT0_e = nc.dram_tensor("fltrt0", [N, HW_], F8, kind="ExternalInput")
    fltrT1_e = nc.dram_tensor("fltrt1", [N, HW_], F8, kind="ExternalInput")
    xt_e = nc.dram_tensor("xt", [P, N // 2], BF16, kind="ExternalInput")
    xtm_e = nc.dram_tensor("xtm", [P, R], BF16, kind="ExternalInput")
    w1i0_e = nc.dram_tensor("w1i0p", [P, P], BF16, kind="ExternalInput")
    w1i1_e = nc.dram_tensor("w1i1p", [P, P], BF16, kind="ExternalInput")
    w2i0_e = nc.dram_tensor("w2i0p", [P, C2], BF16, kind="ExternalInput")
    w2i1_e = nc.dram_tensor("w2i1p", [P, C2], BF16, kind="ExternalInput")
    bi0_e = nc.dram_tensor("bi0d", [P, 1], F32, kind="ExternalInput")
    bi1h_e = nc.dram_tensor("bi1hd", [P, 1], F32, kind="ExternalInput")
    ssum_e = nc.dram_tensor("ssum", [P, P], BF16, kind="ExternalInput")
    out_e = nc.dram_tensor("out", [C, R], F32, kind="ExternalOutput")

    RG = [list(range(NCORES))]
    fltr_halves = [fltrT0_e, fltrT1_e]

    with tile.TileContext(nc) as tc:
        with (
            tc.tile_pool(name="wpool", bufs=1) as wpool,
            tc.tile_pool(name="y0pool", bufs=1) as y0pool,
            tc.tile_pool(name="ygpool", bufs=1) as ygpool,
            tc.tile_pool(name="kpool", bufs=1) as kpool,
            tc.tile_pool(name="fpool", bufs=8) as fpool,
            tc.tile_pool(name="xbpool", bufs=2) as xbpool,
            tc.tile_pool(name="ylpool", bufs=2) as ylpool,
            tc.tile_pool(name="pacc", bufs=4, space="PSUM") as pacc,
            tc.tile_pool(name="psmall", bufs=2, space="PSUM") as psmall,
            tc.tile_pool(name="dram", bufs=8, space="DRAM") as dram,
        ):
            # ---- resident small tensors (scalar ring) + xt (sync ring,
            # ---- ahead of the fltr stream; 4 independent tiles)
            w1i0p = wpool.tile([P, P], BF16)
            nc.scalar.dma_start(w1i0p[:], w1i0_e[:])
            xts = []
            for q in range(4):
                xq = wpool.tile([P, N // 8], BF16, name=f"xt{q}")
                nc.sync.dma_start(xq[:],
                                  xt_e[:, q * (N // 8):(q + 1) * (N // 8)])
                xts.append(xq)

            # dummy warm-up collective: anchors the one-time rendezvous
            # barrier while all cores are still in startup.
            gwin = dram.tile([F, C2], BF16, name="gwin", tag="gwin")
            nc.scalar.dma_start(gwin[:], w1i0p[0:F, 0:C2])
            gwout = dram.tile([NCORES * F, C2], BF16, name="gwout",
                              tag="gwout", addr_space="Shared")
            nc.gpsimd.collective_compute(
                "AllGather", mybir.AluOpType.bypass,
                replica_groups=RG,
                ins=[gwin[:].opt()], outs=[gwout[:].opt()],
            )

            w1i1p = wpool.tile([P, P], BF16)
            nc.scalar.dma_start(w1i1p[:], w1i1_e[:])
            w2i0p = wpool.tile([P, C2], BF16)
            nc.scalar.dma_start(w2i0p[:], w2i0_e[:])
            w2i1p = wpool.tile([P, C2], BF16)
            nc.scalar.dma_start(w2i1p[:], w2i1_e[:])
            bi0d = wpool.tile([P, 1], F32)
            nc.scalar.dma_start(bi0d[:], bi0_e[:])
            bi1hd = wpool.tile([P, 1], F32)
            nc.scalar.dma_start(bi1hd[:], bi1h_e[:])
            ssum = wpool.tile([P, P], BF16)
            nc.scalar.dma_start(ssum[:], ssum_e[:])
            xm = wpool.tile([P, R], BF16)
            nc.scalar.dma_start(xm[:], xtm_e[:])

            y0 = y0pool.tile([P, NKT * C2], BF16, tag="y0")

            def y0_block(g):
                # kt tiles 8g..8g+8 (1024 nodes).  Even/odd 64-node
                # groups run col-tiled on (0,0)/(0,64).
                ps = psmall.tile([P, CW], F32, name="ps0", tag="ps0")
                xq = xts[(g % 8) // 2]
                for kt2 in range(8):
                    kt = g * 8 + kt2
                    hi = kt >= NKT // 2
                    col = (kt % (NKT // 2)) * P - ((g % 8) // 2) * (N // 8)
                    w1rhs = w1i0p[:, C2:P] if hi else w1i0p[:, 0:C2]
                    nc.tensor.matmul(
                        ps[0:C2, kt2 * C2:(kt2 + 1) * C2],
                        xq[:, col:col + C2], w1rhs,
                        start=True, stop=True, tile_position=(0, 0),
                    )
                    nc.tensor.matmul(
                        ps[C2:P, kt2 * C2:(kt2 + 1) * C2],
                        xq[:, col + C2:col + P], w1rhs,
                        start=True, stop=True, tile_position=(0, C2),
                    )
                nc.vector.tensor_copy(y0[:, g * CW:(g + 1) * CW], ps[:])

            # ---- pass-1 state
            p1acc = [None, None]
            kept = {}

            def p1_dma_block(h, ktb):
                if ktb % 4 < 2 and ktb // 4 < JPIN:
                    ft = kpool.tile([P, KB1, HW_], F8, name="ftk",
                                    tag="ftk", bufs=4 * JPIN)
                    kept[(h, ktb)] = ft
                else:
                    ft = fpool.tile([P, KB1, HW_], F8, name="ft", tag="ft")
                nc.sync.dma_start(
                    ft[:],
                    fltr_halves[h][ktb * KB1 * P:(ktb + 1) * KB1 * P, :]
                    .rearrange("(b p) c -> p b c", p=P),
                )
                return ft

            def p1_mms(h, ktb, ft):
                acc = p1acc[h]
                for b in range(KB1):
                    kt = ktb * KB1 + b
                    first = kt == 0
                    yt = y0[:, kt * C2:(kt + 1) * C2]
                    nc.tensor.matmul(acc[0:C2, :], yt, ft[:, b, 0:CW],
                                     start=first, stop=False,
                                     tile_position=(0, 0))
                    nc.tensor.matmul(acc[C2:P, :], yt, ft[:, b, CW:HW_],
                                     start=first, stop=False,
                                     tile_position=(0, C2))

            def p1_epilogue(h):
                acc = p1acc[h]
                # x@w2 term, contracted over zero-padded K=128 (stop)
                nc.tensor.matmul(acc[0:C2, :], w2i0p[:],
                                 xm[:, 2 * h * CW:(2 * h + 1) * CW],
                                 start=False, stop=True,
                                 tile_position=(0, 0))
                nc.tensor.matmul(acc[C2:P, :], w2i0p[:],
                                 xm[:, (2 * h + 1) * CW:(2 * h + 2) * CW],
                                 start=False, stop=True,
                                 tile_position=(0, C2))
                xb1 = xbpool.tile([P, CW], BF16, name="xb1")
                nc.scalar.activation(
                    xb1[0:C2, :], acc[0:C2, :],
                    mybir.ActivationFunctionType.Relu,
                    bias=bi0d[0:C2, :], scale=1.0,
                )
                nc.scalar.activation(
                    xb1[C2:P, :], acc[C2:P, :],
                    mybir.ActivationFunctionType.Relu,
                    bias=bi0d[C2:P, :], scale=1.0,
                )
                psy = psmall.tile([P, CW], F32, name="psy", tag="ps0")
                for g in range(16):
                    w1rhs = w1i1p[:, 0:C2] if g < 8 else w1i1p[:, C2:P]
                    lcol = (g % 8) * C2
                    nc.tensor.matmul(
                        psy[(g % 2) * C2:(g % 2) * C2 + C2,
                            (g // 2) * C2:(g // 2) * C2 + C2],
                        xb1[:, lcol:lcol + C2], w1rhs,
                        start=True, stop=True,
                        tile_position=(0, (g % 2) * C2),
                    )
                y1h = ylpool.tile([P, 8, C2], BF16, name="y1h")
                nc.vector.tensor_copy(
                    y1h[:], psy[:].rearrange("p (t ch) -> p t ch", ch=C2))
                gin = dram.tile([HW_, C2], BF16, name="gin", tag="gin",
                                bufs=2)
                nc.scalar.dma_start(
                    gin[:].rearrange("(t p) ch -> p t ch", p=P), y1h[:],
                )
                gout = dram.tile(
                    [NCORES * HW_, C2], BF16, name="gout", tag="gout",
                    addr_space="Shared", bufs=2,
                )
                nc.gpsimd.collective_compute(
                    "AllGather", mybir.AluOpType.bypass,
                    replica_groups=RG,
                    ins=[gin[:].opt()], outs=[gout[:].opt()],
                )
                gouts.append(gout)

            gouts = []
            yg = [ygpool.tile([P, NCORES * 8 * C2], BF16, name=f"yg{h}",
                              tag=f"yg{h}") for h in range(2)]
            yg_issued = [False] * 2

            def issue_yg(hg):
                if not yg_issued[hg]:
                    nc.scalar.dma_start(
                        yg[hg][:].rearrange("p (b ch) -> p b ch", ch=C2),
                        gouts[hg][:].rearrange("(b p) ch -> p b ch", p=P),
                    )
                    yg_issued[hg] = True

            # ---- pass-2 state
            p2acc = [None, None]   # [chunks 0|1, chunks 2|3]

            def p2_init():
                p2acc[0] = pacc.tile([P, CW], F32, name="p2a", tag="acc")
                p2acc[1] = pacc.tile([P, CW], F32, name="p2b", tag="acc")
                for pair in range(2):
                    for t in range(2):
                        ck = 2 * pair + t
                        nc.tensor.matmul(
                            p2acc[pair][t * C2:(t + 1) * C2, :],
                            w2i1p[:], xm[:, ck * CW:(ck + 1) * CW],
                            start=True, stop=False,
                            tile_position=(0, t * C2),
                        )

            def p2_tile(hg, j, i, ft0, ft1, stop=False):
                b = i % 4
                yt = yg[hg][:, (j * 8 + i) * C2:(j * 8 + i + 1) * C2]
                nc.tensor.matmul(p2acc[0][0:C2, :], yt, ft0[:, b, 0:CW],
                                 start=False, stop=stop,
                                 tile_position=(0, 0))
                nc.tensor.matmul(p2acc[0][C2:P, :], yt, ft0[:, b, CW:HW_],
                                 start=False, stop=stop,
                                 tile_position=(0, C2))
                if ft1 is None:
                    return
                nc.tensor.matmul(p2acc[1][0:C2, :], yt, ft1[:, b, 0:CW],
                                 start=False, stop=stop,
                                 tile_position=(0, 0))
                nc.tensor.matmul(p2acc[1][C2:P, :], yt, ft1[:, b, CW:HW_],
                                 start=False, stop=stop,
                                 tile_position=(0, C2))

            def p2_stream_block(hg, j, blk_i):
                # DMA half-array 0/1 blocks for (j, i in [4*blk_i,+4))
                ktb = j * 4 + 2 * hg + blk_i
                fts = []
                for ha in range(2):
                    ft = kept.get((ha, ktb))
                    if ft is None:
                        ft = fpool.tile([P, KB1, HW_], F8, name="ft2",
                                        tag="ft")
                        nc.sync.dma_start(
                            ft[:],
                            fltr_halves[ha][ktb * KB1 * P:
                                            (ktb + 1) * KB1 * P, :]
                            .rearrange("(b p) c -> p b c", p=P),
                        )
                    fts.append(ft)
                return fts

            def p2_epilogue(pair):
                acc = p2acc[pair]
                xb2 = xbpool.tile([P, CW], BF16, name="xb2")
                for t in range(2):
                    nc.scalar.activation(
                        xb2[t * C2:(t + 1) * C2, :],
                        acc[t * C2:(t + 1) * C2, :],
                        mybir.ActivationFunctionType.Relu,
                        bias=bi1hd[t * C2:(t + 1) * C2, :], scale=0.5,
                    )
                pso = psmall.tile([P, CW], F32, name="pso", tag="ps0")
                for t in range(2):
                    nc.tensor.matmul(
                        pso[t * C2:(t + 1) * C2, :],
                        ssum[:, t * C2:(t + 1) * C2],
                        xb2[:],
                        start=True, stop=True,
                        tile_position=(0, t * C2),
                    )
                oT = xbpool.tile([P, CW], F32, name="oT")
                nc.vector.tensor_copy(oT[:], pso[:])
                for t in range(2):
                    ck = 2 * pair + t
                    nc.scalar.dma_start(
                        out_e[:, ck * CW:(ck + 1) * CW],
                        oT[t * C2:t * C2 + C, :],
                    )

            # ================= emission =================
            # pass-1 half 0 (+ Y0 interleaved)
            p1acc[0] = pacc.tile([P, CW], F32, name="p1a", tag="acc")
            for ktb in range(NBLK):
                if ktb % 2 == 0 and ktb // 2 < 16:
                    y0_block(ktb // 2)
                ft = p1_dma_block(0, ktb)
                p1_mms(0, ktb, ft)
            p1_epilogue(0)

            # pass-1 half 1 with phase-A interleaving
            import os as _os
            no_ilv = _os.environ.get("ARMA_NO_ILV") == "1"
            jilv = 0 if no_ilv else 3
            p1acc[1] = pacc.tile([P, CW], F32, name="p1b", tag="acc")
            for ktb in range(NBLK):
                ft = p1_dma_block(1, ktb)
                p1_mms(1, ktb, ft)
                if no_ilv:
                    continue
                if ktb == 8:
                    issue_yg(0)
                if ktb == 12:
                    p2_init()
                if ktb >= 16 and ktb % 4 == 0 and (ktb - 16) // 4 < 3:
                    j = (ktb - 16) // 4
                    for i in range(8):
                        ktb2 = j * 4 + i // 4
                        p2_tile(0, j, i, kept[(0, ktb2)], kept[(1, ktb2)])
            p1_epilogue(1)
            if no_ilv:
                issue_yg(0)
                p2_init()

            # remaining pinned phase A (j = jilv..JPIN-1)
            for j in range(jilv, JPIN):
                for i in range(8):
                    ktb2 = j * 4 + i // 4
                    p2_tile(0, j, i, kept[(0, ktb2)], kept[(1, ktb2)])
            issue_yg(1)
            # streamed phase A (j = JPIN..7)
            for j in range(JPIN, NCORES):
                for blk_i in range(2):
                    fts = p2_stream_block(0, j, blk_i)
                    for i in range(4 * blk_i, 4 * blk_i + 4):
                        p2_tile(0, j, i, fts[0], fts[1])
            # phase B (gather half 1); last block split for epilogue hiding
            for j in range(NCORES):
                for blk_i in range(2):
                    lastblk = j == NCORES - 1 and blk_i == 1
                    fts = p2_stream_block(1, j, blk_i)
                    if not lastblk:
                        for i in range(4 * blk_i, 4 * blk_i + 4):
                            p2_tile(1, j, i, fts[0], fts[1])
                    else:
                        for i in range(4, 8):
                            # chunks 0,1 first; stop their groups
                            b = i % 4
                            yt = yg[1][:, (j * 8 + i) * C2:
                                       (j * 8 + i + 1) * C2]
                            nc.tensor.matmul(
                                p2acc[0][0:C2, :], yt, fts[0][:, b, 0:CW],
                                start=False, stop=(i == 7),
                                tile_position=(0, 0))
                            nc.tensor.matmul(
                                p2acc[0][C2:P, :], yt, fts[0][:, b, CW:HW_],
                                start=False, stop=(i == 7),
                                tile_position=(0, C2))
                        p2_epilogue(0)
                        for i in range(4, 8):
                            b = i % 4
                            yt = yg[1][:, (j * 8 + i) * C2:
                                       (j * 8 + i + 1) * C2]
                            nc.tensor.matmul(
                                p2acc[1][0:C2, :], yt, fts[1][:, b, 0:CW],
                                start=False, stop=(i == 7),
                                tile_position=(0, 0))
                            nc.tensor.matmul(
                                p2acc[1][C2:P, :], yt, fts[1][:, b, CW:HW_],
                                start=False, stop=(i == 7),
                                tile_position=(0, C2))
                        p2_epilogue(1)

    nc.compile()
    return nc


def kernel(**inputs):
    x = np.ascontiguousarray(np.asarray(inputs["x"], dtype=np.float32))
    fltr = np.ascontiguousarray(np.asarray(inputs["fltr"], dtype=np.float32))

    def cat(a, b, axis=1):
        return np.ascontiguousarray(
            np.concatenate(
                [np.asarray(a, np.float32), np.asarray(b, np.float32)],
                axis=axis,
            )
        )

    f8 = ml_dtypes.float8_e3m4
    bf = ml_dtypes.bfloat16

    # fused conv kernels, descaled by 2^-8 (fp8 fold)
    w1i0f = (cat(inputs["k0i0_w1"], inputs["k1i0_w1"]) / FSCALE)  # [64,64]
    w1i1f = np.zeros((C2, C2), dtype=np.float32)
    w1i1f[0:C, 0:C] = np.asarray(inputs["k0i1_w1"], np.float32)
    w1i1f[C:C2, C:C2] = np.asarray(inputs["k1i1_w1"], np.float32)
    w1i1f = w1i1f / FSCALE

    def dpad(w):  # [[w,0],[0,w]] -> [128, 128]
        o = np.zeros((P, P), dtype=np.float32)
        o[0:C2, 0:C2] = w
        o[C2:P, C2:P] = w
        return o

    w1i0p = np.ascontiguousarray(dpad(w1i0f).astype(bf))
    w1i1p = np.ascontiguousarray(dpad(w1i1f).astype(bf))

    def zpad(w):  # [w; 0] -> [128, 64]
        o = np.zeros((P, C2), dtype=np.float32)
        o[0:C2, :] = w
        return np.ascontiguousarray(o)

    w2i0p = zpad(cat(inputs["k0i0_w2"], inputs["k1i0_w2"])).astype(bf)
    w2i1p = zpad(cat(inputs["k0i1_w2"], inputs["k1i1_w2"])).astype(bf)
    bi0d = np.ascontiguousarray(
        np.tile(cat(inputs["k0i0_b"], inputs["k1i0_b"], axis=0), 2)[:, None])
    bi1hd = np.ascontiguousarray(
        0.5 * np.tile(cat(inputs["k0i1_b"], inputs["k1i1_b"], axis=0),
                      2)[:, None])

    # stack-mean selection: out[m] = xb2[m] + xb2[m+32] for m < 32
    ssum = np.zeros((P, P), dtype=np.float32)
    for m in range(C):
        ssum[m, m] = ssum[m + C, m] = 1.0
        ssum[C2 + m, C2 + m] = ssum[C2 + C + m, C2 + m] = 1.0
    ssum = np.ascontiguousarray(ssum.astype(bf))

    xT = x.T.astype(bf)                      # [64, 16384]
    xt = np.ascontiguousarray(
        np.concatenate([xT[:, 0:N // 2], xT[:, N // 2:N]], axis=0))

    # fp8 E3M4 fltr at rest, transposed per core, scaled by 2^8
    fltrs = (fltr * np.float32(FSCALE)).astype(f8)

    if "nc" not in _CACHE:
        _CACHE["nc"] = _build()
    nc = _CACHE["nc"]

    in_maps = []
    for m in range(NCORES):
        rows = slice(m * R, (m + 1) * R)
        xtm = np.zeros((P, R), dtype=np.float32)
        xtm[0:F, :] = x[rows, :].T
        xtm = xtm.astype(bf)
        in_maps.append({
            "fltrt0": np.ascontiguousarray(fltrs[m * R:m * R + HW_, :].T),
            "fltrt1": np.ascontiguousarray(
                fltrs[m * R + HW_:(m + 1) * R, :].T),
            "xt": xt,
            "xtm": np.ascontiguousarray(xtm),
            "w1i0p": w1i0p, "w1i1p": w1i1p,
            "w2i0p": w2i0p, "w2i1p": w2i1p,
            "bi0d": bi0d, "bi1hd": bi1hd, "ssum": ssum,
        })

    import os
    import time
    trace = os.environ.get("ARMA_TRACE") == "1"
    last_exc = None
    for attempt in range(3):
        try:
            res = run_bass_kernel_spmd(
                nc, in_maps, core_ids=list(range(NCORES)), trace=trace,
            )
            break
        except Exception as e:  # transient NRT device errors: retry
            last_exc = e
            time.sleep(5.0)
    else:
        raise last_exc
    _CACHE["last_results"] = res
    out = np.concatenate(
        [np.asarray(res.results[m]["out"]).T for m in range(NCORES)], axis=0
    )
    return out


# revision 13
# speedup vs baseline: 1.1571x; 1.0278x over previous
"""Distributed ARMAConv kernel for 8 TRN2 NeuronCores (Bass/Tile).

Reference computation (N=16384 nodes, F=64 in-feats, C=32 channels,
K=2 stacks, T=2 iterations):
    for each stack k:  xbar = x
        for i in 0..1: xbar = relu(fltr @ (xbar @ w1) + x @ w2 + b)
    out = mean over stacks                                  -> [N, 32]

Strategy (v2 - 2x column-tiled PE):
  - Row-shard fltr across 8 cores; core m holds fltr[rows_m, :] stored
    TRANSPOSED as two contiguous half-arrays (1 KiB DMA lines), fp8
    E3M4 at rest, pre-scaled by 2^8 (descale folded into w1).
  - Fuse the two ARMA stacks: Y = [xbar@w1_k0 | xbar@w1_k1] is [N,64],
    so fltr streams once per iteration.
  - The stationary operand (Y tile [128,64]) only fills half the
    128-wide PE array.  ALL matmuls run 2x column-tiled (tile_size
    (128,64)): tile (0,0) -> PSUM partitions 0-63, tile (0,64) ->
    64-127, each with its own moving fltr stream.  Measured 519 ns per
    kt-tile (2 concurrent 512-wide fp8 streams) vs 1034 serial -> PE
    ~67us per pass instead of ~110, making the kernel DMA-bound.
  - Every matmul in the kernel keeps tile_size (128,64) (no mode
    switches): K=64 matmuls (Y0 = x@w1, Y1 = relu@w1i1, w2-terms,
    final stack-mean) are zero-padded to K=128, with zeros placed in
    the operand that multiplies the junk rows.  The final stack-mean
    (out = 0.5*(relu_lo + relu_hi)) runs on the PE via a 0/1
    selection stationary instead of partition-shift DMA + DVE add.
  - The x@w2 bias term is accumulated LAST (stop) instead of first,
    so xm's DMA is off the critical path; the first fltr matmul
    carries start=True.
  - SBUF pinning: the pass-1 fltr blocks that pass-2 phase A (gather
    half 0, low core-blocks) consumes are kept resident (JPIN blocks
    per half-array stream); phase-A matmuls for those blocks are
    interleaved into pass-1 half-1's DMA-bound stream, filling PE
    idle slots.  Pass 2 re-reads only the rest.
  - Big fltr DMAs ride the sync-engine HWDGE ring; small/latency DMAs
    ride the scalar-engine ring; collectives keep the gpsimd queue.
    A dummy warm-up collective at t=0 absorbs the one-time rendezvous
    barrier (~60us) that would otherwise delay gather 0.
"""

import numpy as np
import ml_dtypes

import concourse.mybir as mybir
import concourse.tile as tile
from concourse import bacc
from concourse.bass_utils import run_bass_kernel_spmd

N = 16384            # nodes
F = 64               # input features
C = 32               # channels per stack
C2 = 2 * C           # fused channels (2 stacks)
NCORES = 8
R = N // NCORES      # fltr rows per core (2048)
P = 128              # partitions
NKT = N // P         # kt tiles per full pass (128)
HW_ = R // 2         # 1024 output rows per half-array
CW = 512             # output rows per chunk / PSUM accumulator slice
KB1 = 4              # kt tiles per fltr DMA block (512 KiB)
NBLK = NKT // KB1    # 32 blocks per half-array
JPIN = 5             # pin blocks {4j,4j+1: j<JPIN} of both half-arrays
FSCALE = 256.0       # power-of-2 fp8 pre-scale (folded into w1)

F32 = mybir.dt.float32
F32R = mybir.dt.float32r
BF16 = mybir.dt.bfloat16
F8 = mybir.dt.float8e3

_CACHE = {}


PHASES = []


def _mark(nc, label):
    PHASES.append((label, sum(1 for _ in nc.all_instructions())))


def _build():
    nc = bacc.Bacc(
        trn_type="TRN2", target_bir_lowering=False, debug=False,
        num_devices=NCORES,
    )
    fltrT0_e = nc.dram_tensor("fltrt0", [N, HW_], F8, kind="ExternalInput")
    fltrT1_e = nc.dram_tensor("fltrt1", [N, HW_], F8, kind="ExternalInput")
    xt_e = nc.dram_tensor("xt", [P, N // 2], BF16, kind="ExternalInput")
    xtm_e = nc.dram_tensor("xtm", [P, R], BF16, kind="ExternalInput")
    w1i0_e = nc.dram_tensor("w1i0p", [P, P], BF16, kind="ExternalInput")
    w1i1_e = nc.dram_tensor("w1i1p", [P, P], BF16, kind="ExternalInput")
    w2i0_e = nc.dram_tensor("w2i0p", [P, C2], BF16, kind="ExternalInput")
    w2i1_e = nc.dram_tensor("w2i1p", [P, C2], BF16, kind="ExternalInput")
    bi0_e = nc.dram_tensor("bi0d", [P, 1], F32, kind="ExternalInput")
    bi1h_e = nc.dram_tensor("bi1hd", [P, 1], F32, kind="ExternalInput")
    ssum_e = nc.dram_tensor("ssum", [P, P], BF16, kind="ExternalInput")
    out_e = nc.dram_tensor("out", [C, R], F32, kind="ExternalOutput")

    RG = [list(range(NCORES))]
    fltr_halves = [fltrT0_e, fltrT1_e]

    with tile.TileContext(nc) as tc:
        with (
            tc.tile_pool(name="wpool", bufs=1) as wpool,
            tc.tile_pool(name="y0pool", bufs=1) as y0pool,
            tc.tile_pool(name="ygpool", bufs=1) as ygpool,
            tc.tile_pool(name="kpool", bufs=1) as kpool,
            tc.tile_pool(name="fpool", bufs=10) as fpool,
            tc.tile_pool(name="xbpool", bufs=2) as xbpool,
            tc.tile_pool(name="ylpool", bufs=2) as ylpool,
            tc.tile_pool(name="pacc", bufs=4, space="PSUM") as pacc,
            tc.tile_pool(name="psmall", bufs=2, space="PSUM") as psmall,
            tc.tile_pool(name="dram", bufs=8, space="DRAM") as dram,
        ):
            # ---- resident small tensors (scalar ring) + xt (sync ring,
            # ---- ahead of the fltr stream; 4 independent tiles)
            w1i0p = wpool.tile([P, P], BF16)
            nc.scalar.dma_start(w1i0p[:], w1i0_e[:])
            xts = []
            for q in range(4):
                xq = wpool.tile([P, N // 8], BF16, name=f"xt{q}")
                nc.scalar.dma_start(xq[:],
                                    xt_e[:, q * (N // 8):(q + 1) * (N // 8)])
                xts.append(xq)

            # dummy warm-up collective: anchors the one-time rendezvous
            # barrier while all cores are still in startup.
            gwin = dram.tile([F, C2], BF16, name="gwin", tag="gwin")
            nc.scalar.dma_start(gwin[:], w1i0p[0:F, 0:C2])
            gwout = dram.tile([NCORES * F, C2], BF16, name="gwout",
                              tag="gwout", addr_space="Shared")
            nc.gpsimd.collective_compute(
                "AllGather", mybir.AluOpType.bypass,
                replica_groups=RG,
                ins=[gwin[:].opt()], outs=[gwout[:].opt()],
            )

            w1i1p = wpool.tile([P, P], BF16)
            nc.scalar.dma_start(w1i1p[:], w1i1_e[:])
            w2i0p = wpool.tile([P, C2], BF16)
            nc.scalar.dma_start(w2i0p[:], w2i0_e[:])
            w2i1p = wpool.tile([P, C2], BF16)
            nc.scalar.dma_start(w2i1p[:], w2i1_e[:])
            bi0d = wpool.tile([P, 1], F32)
            nc.scalar.dma_start(bi0d[:], bi0_e[:])
            bi1hd = wpool.tile([P, 1], F32)
            nc.scalar.dma_start(bi1hd[:], bi1h_e[:])
            ssum = wpool.tile([P, P], BF16)
            nc.scalar.dma_start(ssum[:], ssum_e[:])
            xm = wpool.tile([P, R], BF16)
            nc.scalar.dma_start(xm[:], xtm_e[:])

            y0 = y0pool.tile([P, NKT * C2], BF16, tag="y0")

            def y0_block(b):
                # twin-node: one N=128 matmul computes Y0 for a 64-node
                # lo-half group (xt rows 0-63 x w1i0p cols 0:64) AND its
                # hi-half twin (rows 64-127 x cols 64:128).  Block b
                # covers lo nodes [512b, 512b+512) = lo kt 4b..4b+4 and
                # their twins kt 64+4b..; psum placement makes the
                # evacuation one contiguous [128, 512] copy.
                ps = psmall.tile([P, 4, P], F32, name="ps0", tag="ps0")
                q4 = b // 4
                base = b * 512 - q4 * (N // 8)
                xq = xts[q4]
                for q in range(8):
                    col = base + q * C2
                    nc.tensor.matmul(
                        ps[(q % 2) * C2:(q % 2) * C2 + C2, q // 2, :],
                        xq[:, col:col + C2], w1i0p[:],
                        start=True, stop=True,
                        tile_position=(0, (q % 2) * C2),
                    )
                nc.vector.tensor_copy(
                    y0[:, b * CW:(b + 1) * CW],
                    ps[:].rearrange("p u c -> p (u c)"))

            # ---- pass-1 state
            p1acc = [None, None]
            kept = {}

            def p1_dma_block(h, ktb):
                if ktb % 4 < 2 and ktb // 4 < JPIN:
                    ft = kpool.tile([P, KB1, HW_], F8, name="ftk",
                                    tag="ftk", bufs=4 * JPIN)
                    kept[(h, ktb)] = ft
                else:
                    ft = fpool.tile([P, KB1, HW_], F8, name="ft", tag="ft")
                nc.sync.dma_start(
                    ft[:],
                    fltr_halves[h][ktb * KB1 * P:(ktb + 1) * KB1 * P, :]
                    .rearrange("(b p) c -> p b c", p=P),
                )
                return ft

            def y0sl(kt):
                if kt < NKT // 2:
                    return y0[:, kt * P:kt * P + C2]
                return y0[:, (kt - NKT // 2) * P + C2:(kt - NKT // 2) * P + P]

            def p1_mms(h, ktb, ft):
                acc = p1acc[h]
                for b in range(KB1):
                    kt = ktb * KB1 + b
                    first = kt == 0
                    yt = y0sl(kt)
                    nc.tensor.matmul(acc[0:C2, :], yt, ft[:, b, 0:CW],
                                     start=first, stop=False,
                                     tile_position=(0, 0))
                    nc.tensor.matmul(acc[C2:P, :], yt, ft[:, b, CW:HW_],
                                     start=first, stop=False,
                                     tile_position=(0, C2))

            def p1_epilogue(h):
                acc = p1acc[h]
                # x@w2 term, contracted over zero-padded K=128 (stop)
                nc.tensor.matmul(acc[0:C2, :], w2i0p[:],
                                 xm[:, 2 * h * CW:(2 * h + 1) * CW],
                                 start=False, stop=True,
                                 tile_position=(0, 0))
                nc.tensor.matmul(acc[C2:P, :], w2i0p[:],
                                 xm[:, (2 * h + 1) * CW:(2 * h + 2) * CW],
                                 start=False, stop=True,
                                 tile_position=(0, C2))
                xb1 = xbpool.tile([P, CW], BF16, name="xb1")
                nc.scalar.activation(
                    xb1[0:C2, :], acc[0:C2, :],
                    mybir.ActivationFunctionType.Relu,
                    bias=bi0d[0:C2, :], scale=1.0,
                )
                nc.scalar.activation(
                    xb1[C2:P, :], acc[C2:P, :],
                    mybir.ActivationFunctionType.Relu,
                    bias=bi0d[C2:P, :], scale=1.0,
                )
                psy = psmall.tile([P, 4, P], F32, name="psy", tag="ps0")
                for g in range(8):
                    nc.tensor.matmul(
                        psy[(g % 2) * C2:(g % 2) * C2 + C2, g // 2, :],
                        xb1[:, g * C2:(g + 1) * C2], w1i1p[:],
                        start=True, stop=True,
                        tile_position=(0, (g % 2) * C2),
                    )
                y1h = ylpool.tile([P, 8, C2], BF16, name="y1h")
                nc.vector.tensor_copy(y1h[:, 0:4, :], psy[:, :, 0:C2])
                nc.vector.tensor_copy(y1h[:, 4:8, :], psy[:, :, C2:P])
                # p-major gather payload: per-partition-contiguous 1 KiB
                gin = dram.tile([P, 8 * C2], BF16, name="gin", tag="gin",
                                bufs=2)
                nc.scalar.dma_start(
                    gin[:], y1h[:].rearrange("p t ch -> p (t ch)"))
                gout = dram.tile(
                    [NCORES * P, 8 * C2], BF16, name="gout", tag="gout",
                    addr_space="Shared", bufs=2,
                )
                nc.gpsimd.collective_compute(
                    "AllGather", mybir.AluOpType.bypass,
                    replica_groups=RG,
                    ins=[gin[:].opt()], outs=[gout[:].opt()],
                )
                gouts.append(gout)

            gouts = []
            yg = [ygpool.tile([P, NCORES * 8 * C2], BF16, name=f"yg{h}",
                              tag=f"yg{h}") for h in range(2)]
            yg_issued = [False] * 2

            def issue_yg(hg):
                if not yg_issued[hg]:
                    nc.scalar.dma_start(
                        yg[hg][:].rearrange("p (m c) -> p m c", c=8 * C2),
                        gouts[hg][:].rearrange("(m p) c -> p m c", p=P),
                    )
                    yg_issued[hg] = True

            # ---- pass-2 state
            p2acc = [None, None]   # [chunks 0|1, chunks 2|3]

            def p2_init():
                p2acc[0] = pacc.tile([P, CW], F32, name="p2a", tag="acc")
                p2acc[1] = pacc.tile([P, CW], F32, name="p2b", tag="acc")
                for pair in range(2):
                    for t in range(2):
                        ck = 2 * pair + t
                        nc.tensor.matmul(
                            p2acc[pair][t * C2:(t + 1) * C2, :],
                            w2i1p[:], xm[:, ck * CW:(ck + 1) * CW],
                            start=True, stop=False,
                            tile_position=(0, t * C2),
                        )

            def p2_tile(hg, j, i, ft0, ft1, stop=False):
                b = i % 4
                yt = yg[hg][:, (j * 8 + i) * C2:(j * 8 + i + 1) * C2]
                nc.tensor.matmul(p2acc[0][0:C2, :], yt, ft0[:, b, 0:CW],
                                 start=False, stop=stop,
                                 tile_position=(0, 0))
                nc.tensor.matmul(p2acc[0][C2:P, :], yt, ft0[:, b, CW:HW_],
                                 start=False, stop=stop,
                                 tile_position=(0, C2))
                if ft1 is None:
                    return
                nc.tensor.matmul(p2acc[1][0:C2, :], yt, ft1[:, b, 0:CW],
                                 start=False, stop=stop,
                                 tile_position=(0, 0))
                nc.tensor.matmul(p2acc[1][C2:P, :], yt, ft1[:, b, CW:HW_],
                                 start=False, stop=stop,
                                 tile_position=(0, C2))

            def p2_stream_block(hg, j, blk_i):
                # DMA half-array 0/1 blocks for (j, i in [4*blk_i,+4))
                ktb = j * 4 + 2 * hg + blk_i
                fts = []
                for ha in range(2):
                    ft = kept.get((ha, ktb))
                    if ft is None:
                        ft = fpool.tile([P, KB1, HW_], F8, name="ft2",
                                        tag="ft")
                        nc.sync.dma_start(
                            ft[:],
                            fltr_halves[ha][ktb * KB1 * P:
                                            (ktb + 1) * KB1 * P, :]
                            .rearrange("(b p) c -> p b c", p=P),
                        )
                    fts.append(ft)
                return fts

            def p2_epilogue(pair):
                acc = p2acc[pair]
                xb2 = xbpool.tile([P, CW], BF16, name="xb2")
                for t in range(2):
                    nc.scalar.activation(
                        xb2[t * C2:(t + 1) * C2, :],
                        acc[t * C2:(t + 1) * C2, :],
                        mybir.ActivationFunctionType.Relu,
                        bias=bi1hd[t * C2:(t + 1) * C2, :], scale=0.5,
                    )
                pso = psmall.tile([P, CW], F32, name="pso", tag="ps0")
                for t in range(2):
                    nc.tensor.matmul(
                        pso[t * C2:(t + 1) * C2, :],
                        ssum[:, t * C2:(t + 1) * C2],
                        xb2[:],
                        start=True, stop=True,
                        tile_position=(0, t * C2),
                    )
                oT = xbpool.tile([P, CW], F32, name="oT")
                nc.vector.tensor_copy(oT[:], pso[:])
                for t in range(2):
                    ck = 2 * pair + t
                    nc.scalar.dma_start(
                        out_e[:, ck * CW:(ck + 1) * CW],
                        oT[t * C2:t * C2 + C, :],
                    )

            # ================= emission =================
            # pass-1 half 0 (+ Y0 interleaved)
            _mark(nc, "p1h0_start")
            p1acc[0] = pacc.tile([P, CW], F32, name="p1a", tag="acc")
            for ktb in range(NBLK):
                if ktb < 16:
                    y0_block(ktb)
                ft = p1_dma_block(0, ktb)
                p1_mms(0, ktb, ft)
            _mark(nc, "p1h0_end")
            p1_epilogue(0)
            _mark(nc, "epi0_end")

            # pass-1 half 1; yg0 load + p2 acc init issued mid-half so
            # phase A can start the moment half-1's matmuls finish
            import os as _os
            ilv = _os.environ.get("ARMA_ILV") == "1"
            jilv = 3 if ilv else 0
            p1acc[1] = pacc.tile([P, CW], F32, name="p1b", tag="acc")
            for ktb in range(NBLK):
                ft = p1_dma_block(1, ktb)
                p1_mms(1, ktb, ft)
                if ktb == 8:
                    issue_yg(0)
                if ktb == 12:
                    p2_init()
                if ilv and ktb >= 16 and ktb % 4 == 0 and (ktb - 16) // 4 < 3:
                    j = (ktb - 16) // 4
                    for i in range(8):
                        ktb2 = j * 4 + i // 4
                        p2_tile(0, j, i, kept[(0, ktb2)], kept[(1, ktb2)])
            _mark(nc, "p1h1_end")
            p1_epilogue(1)
            _mark(nc, "epi1_end")
            issue_yg(1)

            # pinned phase A (j = jilv..JPIN-1)
            for j in range(jilv, JPIN):
                for i in range(8):
                    ktb2 = j * 4 + i // 4
                    p2_tile(0, j, i, kept[(0, ktb2)], kept[(1, ktb2)])
            _mark(nc, "phA_pinned_end")
            # streamed phase A (j = JPIN..7)
            for j in range(JPIN, NCORES):
                for blk_i in range(2):
                    fts = p2_stream_block(0, j, blk_i)
                    for i in range(4 * blk_i, 4 * blk_i + 4):
                        p2_tile(0, j, i, fts[0], fts[1])
            _mark(nc, "phA_end")
            # phase B (gather half 1); last block split for epilogue hiding
            for j in range(NCORES):
                for blk_i in range(2):
                    lastblk = j == NCORES - 1 and blk_i == 1
                    fts = p2_stream_block(1, j, blk_i)
                    if not lastblk:
                        for i in range(4 * blk_i, 4 * blk_i + 4):
                            p2_tile(1, j, i, fts[0], fts[1])
                    else:
                        for i in range(4, 8):
                            # chunks 0,1 first; stop their groups
                            b = i % 4
                            yt = yg[1][:, (j * 8 + i) * C2:
                                       (j * 8 + i + 1) * C2]
                            nc.tensor.matmul(
                                p2acc[0][0:C2, :], yt, fts[0][:, b, 0:CW],
                                start=False, stop=(i == 7),
                                tile_position=(0, 0))
                            nc.tensor.matmul(
                                p2acc[0][C2:P, :], yt, fts[0][:, b, CW:HW_],
                                start=False, stop=(i == 7),
                                tile_position=(0, C2))
                        p2_epilogue(0)
                        for i in range(4, 8):
                            b = i % 4
                            yt = yg[1][:, (j * 8 + i) * C2:
                                       (j * 8 + i + 1) * C2]
                            nc.tensor.matmul(
                                p2acc[1][0:C2, :], yt, fts[1][:, b, 0:CW],
                                start=False, stop=(i == 7),
                                tile_position=(0, 0))
                            nc.tensor.matmul(
                                p2acc[1][C2:P, :], yt, fts[1][:, b, CW:HW_],
                                start=False, stop=(i == 7),
                                tile_position=(0, C2))
                        p2_epilogue(1)

    nc.compile()
    return nc


def kernel(**inputs):
    x = np.ascontiguousarray(np.asarray(inputs["x"], dtype=np.float32))
    fltr = np.ascontiguousarray(np.asarray(inputs["fltr"], dtype=np.float32))

    def cat(a, b, axis=1):
        return np.ascontiguousarray(
            np.concatenate(
                [np.asarray(a, np.float32), np.asarray(b, np.float32)],
                axis=axis,
            )
        )

    f8 = ml_dtypes.float8_e3m4
    bf = ml_dtypes.bfloat16

    # fused conv kernels, descaled by 2^-8 (fp8 fold)
    w1i0f = (cat(inputs["k0i0_w1"], inputs["k1i0_w1"]) / FSCALE)  # [64,64]
    w1i1f = np.zeros((C2, C2), dtype=np.float32)
    w1i1f[0:C, 0:C] = np.asarray(inputs["k0i1_w1"], np.float32)
    w1i1f[C:C2, C:C2] = np.asarray(inputs["k1i1_w1"], np.float32)
    w1i1f = w1i1f / FSCALE

    def dpad(w):  # [[w,0],[0,w]] -> [128, 128]
        o = np.zeros((P, P), dtype=np.float32)
        o[0:C2, 0:C2] = w
        o[C2:P, C2:P] = w
        return o

    w1i0p = np.ascontiguousarray(dpad(w1i0f).astype(bf))
    w1i1p = np.ascontiguousarray(dpad(w1i1f).astype(bf))

    def zpad(w):  # [w; 0] -> [128, 64]
        o = np.zeros((P, C2), dtype=np.float32)
        o[0:C2, :] = w
        return np.ascontiguousarray(o)

    w2i0p = zpad(cat(inputs["k0i0_w2"], inputs["k1i0_w2"])).astype(bf)
    w2i1p = zpad(cat(inputs["k0i1_w2"], inputs["k1i1_w2"])).astype(bf)
    bi0d = np.ascontiguousarray(
        np.tile(cat(inputs["k0i0_b"], inputs["k1i0_b"], axis=0), 2)[:, None])
    bi1hd = np.ascontiguousarray(
        0.5 * np.tile(cat(inputs["k0i1_b"], inputs["k1i1_b"], axis=0),
                      2)[:, None])

    # stack-mean selection: out[m] = xb2[m] + xb2[m+32] for m < 32
    ssum = np.zeros((P, P), dtype=np.float32)
    for m in range(C):
        ssum[m, m] = ssum[m + C, m] = 1.0
        ssum[C2 + m, C2 + m] = ssum[C2 + C + m, C2 + m] = 1.0
    ssum = np.ascontiguousarray(ssum.astype(bf))

    xT = x.T.astype(bf)                      # [64, 16384]
    xt = np.ascontiguousarray(
        np.concatenate([xT[:, 0:N // 2], xT[:, N // 2:N]], axis=0))

    # fp8 E3M4 fltr at rest, transposed per core, scaled by 2^8
    fltrs = (fltr * np.float32(FSCALE)).astype(f8)

    if "nc" not in _CACHE:
        _CACHE["nc"] = _build()
    nc = _CACHE["nc"]

    in_maps = []
    for m in range(NCORES):
        rows = slice(m * R, (m + 1) * R)
        xtm = np.zeros((P, R), dtype=np.float32)
        xtm[0:F, :] = x[rows, :].T
        xtm = xtm.astype(bf)
        in_maps.append({
            "fltrt0": np.ascontiguousarray(fltrs[m * R:m * R + HW_, :].T),
            "fltrt1": np.ascontiguousarray(
                fltrs[m * R + HW_:(m + 1) * R, :].T),
            "xt": xt,
            "xtm": np.ascontiguousarray(xtm),
            "w1i0p": w1i0p, "w1i1p": w1i1p,
            "w2i0p": w2i0p, "w2i1p": w2i1p,
            "bi0d": bi0d, "bi1hd": bi1hd, "ssum": ssum,
        })

    import os
    import time
    trace = os.environ.get("ARMA_TRACE") == "1"
    last_exc = None
    for attempt in range(3):
        try:
            res = run_bass_kernel_spmd(
                nc, in_maps, core_ids=list(range(NCORES)), trace=trace,
            )
            break
        except Exception as e:  # transient NRT device errors: retry
            last_exc = e
            time.sleep(5.0)
    else:
        raise last_exc
    _CACHE["last_results"] = res
    out = np.concatenate(
        [np.asarray(res.results[m]["out"]).T for m in range(NCORES)], axis=0
    )
    return out


# revision 14
# speedup vs baseline: 1.1833x; 1.0227x over previous
"""Distributed ARMAConv kernel for 8 TRN2 NeuronCores (Bass/Tile).

Reference computation (N=16384 nodes, F=64 in-feats, C=32 channels,
K=2 stacks, T=2 iterations):
    for each stack k:  xbar = x
        for i in 0..1: xbar = relu(fltr @ (xbar @ w1) + x @ w2 + b)
    out = mean over stacks                                  -> [N, 32]

Strategy (v2 - 2x column-tiled PE):
  - Row-shard fltr across 8 cores; core m holds fltr[rows_m, :] stored
    TRANSPOSED as two contiguous half-arrays (1 KiB DMA lines), fp8
    E3M4 at rest, pre-scaled by 2^8 (descale folded into w1).
  - Fuse the two ARMA stacks: Y = [xbar@w1_k0 | xbar@w1_k1] is [N,64],
    so fltr streams once per iteration.
  - The stationary operand (Y tile [128,64]) only fills half the
    128-wide PE array.  ALL matmuls run 2x column-tiled (tile_size
    (128,64)): tile (0,0) -> PSUM partitions 0-63, tile (0,64) ->
    64-127, each with its own moving fltr stream.  Measured 519 ns per
    kt-tile (2 concurrent 512-wide fp8 streams) vs 1034 serial -> PE
    ~67us per pass instead of ~110, making the kernel DMA-bound.
  - Every matmul in the kernel keeps tile_size (128,64) (no mode
    switches): K=64 matmuls (Y0 = x@w1, Y1 = relu@w1i1, w2-terms,
    final stack-mean) are zero-padded to K=128, with zeros placed in
    the operand that multiplies the junk rows.  The final stack-mean
    (out = 0.5*(relu_lo + relu_hi)) runs on the PE via a 0/1
    selection stationary instead of partition-shift DMA + DVE add.
  - The x@w2 bias term is accumulated LAST (stop) instead of first,
    so xm's DMA is off the critical path; the first fltr matmul
    carries start=True.
  - SBUF pinning: the pass-1 fltr blocks that pass-2 phase A (gather
    half 0, low core-blocks) consumes are kept resident (JPIN blocks
    per half-array stream); phase-A matmuls for those blocks are
    interleaved into pass-1 half-1's DMA-bound stream, filling PE
    idle slots.  Pass 2 re-reads only the rest.
  - Big fltr DMAs ride the sync-engine HWDGE ring; small/latency DMAs
    ride the scalar-engine ring; collectives keep the gpsimd queue.
    A dummy warm-up collective at t=0 absorbs the one-time rendezvous
    barrier (~60us) that would otherwise delay gather 0.
"""

import numpy as np
import ml_dtypes

import concourse.mybir as mybir
import concourse.tile as tile
from concourse import bacc
from concourse.bass_utils import run_bass_kernel_spmd

N = 16384            # nodes
F = 64               # input features
C = 32               # channels per stack
C2 = 2 * C           # fused channels (2 stacks)
NCORES = 8
R = N // NCORES      # fltr rows per core (2048)
P = 128              # partitions
NKT = N // P         # kt tiles per full pass (128)
HW_ = R // 2         # 1024 output rows per half-array
CW = 512             # output rows per chunk / PSUM accumulator slice
KB1 = 4              # kt tiles per fltr DMA block (512 KiB)
NBLK = NKT // KB1    # 32 blocks per half-array
JPIN = 6             # pin blocks {4j,4j+1: j<JPIN} of both half-arrays
FSCALE = 256.0       # power-of-2 fp8 pre-scale (folded into w1)

F32 = mybir.dt.float32
F32R = mybir.dt.float32r
BF16 = mybir.dt.bfloat16
F8 = mybir.dt.float8e3

_CACHE = {}


PHASES = []


def _mark(nc, label):
    PHASES.append((label, sum(1 for _ in nc.all_instructions())))


def _build():
    nc = bacc.Bacc(
        trn_type="TRN2", target_bir_lowering=False, debug=False,
        num_devices=NCORES,
    )
    fltrT0_e = nc.dram_tensor("fltrt0", [N, HW_], F8, kind="ExternalInput")
    fltrT1_e = nc.dram_tensor("fltrt1", [N, HW_], F8, kind="ExternalInput")
    xt_e = nc.dram_tensor("xt", [P, N // 2], BF16, kind="ExternalInput")
    xtm_e = nc.dram_tensor("xtm", [P, R], BF16, kind="ExternalInput")
    w1i0_e = nc.dram_tensor("w1i0p", [P, P], BF16, kind="ExternalInput")
    w1i1_e = nc.dram_tensor("w1i1p", [P, P], BF16, kind="ExternalInput")
    w2i0_e = nc.dram_tensor("w2i0p", [P, C2], BF16, kind="ExternalInput")
    w2i1_e = nc.dram_tensor("w2i1p", [P, C2], BF16, kind="ExternalInput")
    bi0_e = nc.dram_tensor("bi0d", [P, 1], F32, kind="ExternalInput")
    bi1h_e = nc.dram_tensor("bi1hd", [P, 1], F32, kind="ExternalInput")
    ssum_e = nc.dram_tensor("ssum", [P, P], BF16, kind="ExternalInput")
    out_e = nc.dram_tensor("out", [C, R], F32, kind="ExternalOutput")

    RG = [list(range(NCORES))]
    fltr_halves = [fltrT0_e, fltrT1_e]

    with tile.TileContext(nc) as tc:
        with (
            tc.tile_pool(name="wpool", bufs=1) as wpool,
            tc.tile_pool(name="y0pool", bufs=1) as y0pool,
            tc.tile_pool(name="xtq", bufs=4) as xtqpool,
            tc.tile_pool(name="kpool", bufs=1) as kpool,
            tc.tile_pool(name="fpool", bufs=8) as fpool,
            tc.tile_pool(name="xbpool", bufs=2) as xbpool,
            tc.tile_pool(name="ylpool", bufs=2) as ylpool,
            tc.tile_pool(name="pacc", bufs=4, space="PSUM") as pacc,
            tc.tile_pool(name="psmall", bufs=2, space="PSUM") as psmall,
            tc.tile_pool(name="dram", bufs=8, space="DRAM") as dram,
        ):
            # ---- resident small tensors (scalar ring) + xt (sync ring,
            # ---- ahead of the fltr stream; 4 independent tiles)
            w1i0p = wpool.tile([P, P], BF16)
            nc.scalar.dma_start(w1i0p[:], w1i0_e[:])
            xts = []
            for q in range(4):
                xq = xtqpool.tile([P, N // 8], BF16, name=f"xt{q}",
                                  tag="xtq")
                nc.scalar.dma_start(xq[:],
                                    xt_e[:, q * (N // 8):(q + 1) * (N // 8)])
                xts.append(xq)

            # dummy warm-up collective: anchors the one-time rendezvous
            # barrier while all cores are still in startup.
            gwin = dram.tile([F, C2], BF16, name="gwin", tag="gwin")
            nc.scalar.dma_start(gwin[:], w1i0p[0:F, 0:C2])
            gwout = dram.tile([NCORES * F, C2], BF16, name="gwout",
                              tag="gwout", addr_space="Shared")
            nc.gpsimd.collective_compute(
                "AllGather", mybir.AluOpType.bypass,
                replica_groups=RG,
                ins=[gwin[:].opt()], outs=[gwout[:].opt()],
            )

            w1i1p = wpool.tile([P, P], BF16)
            nc.scalar.dma_start(w1i1p[:], w1i1_e[:])
            w2i0p = wpool.tile([P, C2], BF16)
            nc.scalar.dma_start(w2i0p[:], w2i0_e[:])
            w2i1p = wpool.tile([P, C2], BF16)
            nc.scalar.dma_start(w2i1p[:], w2i1_e[:])
            bi0d = wpool.tile([P, 1], F32)
            nc.scalar.dma_start(bi0d[:], bi0_e[:])
            bi1hd = wpool.tile([P, 1], F32)
            nc.scalar.dma_start(bi1hd[:], bi1h_e[:])
            ssum = wpool.tile([P, P], BF16)
            nc.scalar.dma_start(ssum[:], ssum_e[:])
            xm = wpool.tile([P, R], BF16)
            nc.scalar.dma_start(xm[:], xtm_e[:])

            y0 = y0pool.tile([P, NKT * C2], BF16, tag="y0")

            def y0_block(b):
                # twin-node: one N=128 matmul computes Y0 for a 64-node
                # lo-half group (xt rows 0-63 x w1i0p cols 0:64) AND its
                # hi-half twin (rows 64-127 x cols 64:128).  Block b
                # covers lo nodes [512b, 512b+512) = lo kt 4b..4b+4 and
                # their twins kt 64+4b..; psum placement makes the
                # evacuation one contiguous [128, 512] copy.
                ps = psmall.tile([P, 4, P], F32, name="ps0", tag="ps0")
                q4 = b // 4
                base = b * 512 - q4 * (N // 8)
                xq = xts[q4]
                for q in range(8):
                    col = base + q * C2
                    nc.tensor.matmul(
                        ps[(q % 2) * C2:(q % 2) * C2 + C2, q // 2, :],
                        xq[:, col:col + C2], w1i0p[:],
                        start=True, stop=True,
                        tile_position=(0, (q % 2) * C2),
                    )
                nc.vector.tensor_copy(
                    y0[:, b * CW:(b + 1) * CW],
                    ps[:].rearrange("p u c -> p (u c)"))

            # ---- pass-1 state
            p1acc = [None, None]
            kept = {}

            def p1_dma_block(h, ktb):
                if ktb % 4 < 2 and ktb // 4 < JPIN:
                    ft = kpool.tile([P, KB1, HW_], F8, name="ftk",
                                    tag="ftk", bufs=4 * JPIN)
                    kept[(h, ktb)] = ft
                else:
                    ft = fpool.tile([P, KB1, HW_], F8, name="ft", tag="ft")
                nc.sync.dma_start(
                    ft[:],
                    fltr_halves[h][ktb * KB1 * P:(ktb + 1) * KB1 * P, :]
                    .rearrange("(b p) c -> p b c", p=P),
                )
                return ft

            def y0sl(kt):
                if kt < NKT // 2:
                    return y0[:, kt * P:kt * P + C2]
                return y0[:, (kt - NKT // 2) * P + C2:(kt - NKT // 2) * P + P]

            def p1_mms(h, ktb, ft):
                acc = p1acc[h]
                for b in range(KB1):
                    kt = ktb * KB1 + b
                    first = kt == 0
                    yt = y0sl(kt)
                    nc.tensor.matmul(acc[0:C2, :], yt, ft[:, b, 0:CW],
                                     start=first, stop=False,
                                     tile_position=(0, 0))
                    nc.tensor.matmul(acc[C2:P, :], yt, ft[:, b, CW:HW_],
                                     start=first, stop=False,
                                     tile_position=(0, C2))

            def p1_epilogue(h):
                ctx_p = tc.high_priority(offset=600)
                ctx_p.__enter__()
                acc = p1acc[h]
                # x@w2 term, contracted over zero-padded K=128 (stop)
                nc.tensor.matmul(acc[0:C2, :], w2i0p[:],
                                 xm[:, 2 * h * CW:(2 * h + 1) * CW],
                                 start=False, stop=True,
                                 tile_position=(0, 0))
                nc.tensor.matmul(acc[C2:P, :], w2i0p[:],
                                 xm[:, (2 * h + 1) * CW:(2 * h + 2) * CW],
                                 start=False, stop=True,
                                 tile_position=(0, C2))
                xb1 = xbpool.tile([P, CW], BF16, name="xb1")
                nc.scalar.activation(
                    xb1[0:C2, :], acc[0:C2, :],
                    mybir.ActivationFunctionType.Relu,
                    bias=bi0d[0:C2, :], scale=1.0,
                )
                nc.scalar.activation(
                    xb1[C2:P, :], acc[C2:P, :],
                    mybir.ActivationFunctionType.Relu,
                    bias=bi0d[C2:P, :], scale=1.0,
                )
                psy = psmall.tile([P, 4, P], F32, name="psy", tag="ps0")
                for g in range(8):
                    nc.tensor.matmul(
                        psy[(g % 2) * C2:(g % 2) * C2 + C2, g // 2, :],
                        xb1[:, g * C2:(g + 1) * C2], w1i1p[:],
                        start=True, stop=True,
                        tile_position=(0, (g % 2) * C2),
                    )
                y1h = ylpool.tile([P, 8, C2], BF16, name="y1h")
                nc.vector.tensor_copy(y1h[:, 0:4, :], psy[:, :, 0:C2])
                nc.vector.tensor_copy(y1h[:, 4:8, :], psy[:, :, C2:P])
                # p-major gather payload: per-partition-contiguous 1 KiB
                gin = dram.tile([P, 8 * C2], BF16, name="gin", tag="gin",
                                bufs=2)
                nc.scalar.dma_start(
                    gin[:], y1h[:].rearrange("p t ch -> p (t ch)"))
                gout = dram.tile(
                    [NCORES * P, 8 * C2], BF16, name="gout", tag="gout",
                    addr_space="Shared", bufs=2,
                )
                nc.gpsimd.collective_compute(
                    "AllGather", mybir.AluOpType.bypass,
                    replica_groups=RG,
                    ins=[gin[:].opt()], outs=[gout[:].opt()],
                )
                gouts.append(gout)
                ctx_p.__exit__(None, None, None)

            gouts = []
            ygt = [None] * 4
            yg_issued = [False] * 2

            def issue_yg(hg):
                if yg_issued[hg]:
                    return
                with tc.high_priority(offset=600):
                    for hh in range(2):
                        t = xtqpool.tile([P, N // 8], BF16,
                                         name=f"yg{hg}{hh}", tag="xtq")
                        ygt[hg * 2 + hh] = t
                        nc.scalar.dma_start(
                            t[:].rearrange("p (m c) -> p m c", c=8 * C2),
                            gouts[hg][hh * 4 * P:(hh * 4 + 4) * P, :]
                            .rearrange("(m p) c -> p m c", p=P),
                        )
                yg_issued[hg] = True

            # ---- pass-2 state
            p2acc = [None, None]   # [chunks 0|1, chunks 2|3]

            def p2_init():
                p2acc[0] = pacc.tile([P, CW], F32, name="p2a", tag="acc")
                p2acc[1] = pacc.tile([P, CW], F32, name="p2b", tag="acc")
                for pair in range(2):
                    for t in range(2):
                        ck = 2 * pair + t
                        nc.tensor.matmul(
                            p2acc[pair][t * C2:(t + 1) * C2, :],
                            w2i1p[:], xm[:, ck * CW:(ck + 1) * CW],
                            start=True, stop=False,
                            tile_position=(0, t * C2),
                        )

            def p2_tile(hg, j, i, ft0, ft1, stop=False):
                b = i % 4
                yt = ygt[hg * 2 + j // 4][
                    :, ((j % 4) * 8 + i) * C2:((j % 4) * 8 + i + 1) * C2]
                nc.tensor.matmul(p2acc[0][0:C2, :], yt, ft0[:, b, 0:CW],
                                 start=False, stop=stop,
                                 tile_position=(0, 0))
                nc.tensor.matmul(p2acc[0][C2:P, :], yt, ft0[:, b, CW:HW_],
                                 start=False, stop=stop,
                                 tile_position=(0, C2))
                if ft1 is None:
                    return
                nc.tensor.matmul(p2acc[1][0:C2, :], yt, ft1[:, b, 0:CW],
                                 start=False, stop=stop,
                                 tile_position=(0, 0))
                nc.tensor.matmul(p2acc[1][C2:P, :], yt, ft1[:, b, CW:HW_],
                                 start=False, stop=stop,
                                 tile_position=(0, C2))

            def p2_stream_block(hg, j, blk_i):
                # DMA half-array 0/1 blocks for (j, i in [4*blk_i,+4))
                ktb = j * 4 + 2 * hg + blk_i
                fts = []
                for ha in range(2):
                    ft = kept.get((ha, ktb))
                    if ft is None:
                        ft = fpool.tile([P, KB1, HW_], F8, name="ft2",
                                        tag="ft")
                        nc.sync.dma_start(
                            ft[:],
                            fltr_halves[ha][ktb * KB1 * P:
                                            (ktb + 1) * KB1 * P, :]
                            .rearrange("(b p) c -> p b c", p=P),
                        )
                    fts.append(ft)
                return fts

            def p2_epilogue(pair):
                ctx_p = tc.high_priority(offset=600)
                ctx_p.__enter__()
                acc = p2acc[pair]
                xb2 = xbpool.tile([P, CW], BF16, name="xb2")
                for t in range(2):
                    nc.scalar.activation(
                        xb2[t * C2:(t + 1) * C2, :],
                        acc[t * C2:(t + 1) * C2, :],
                        mybir.ActivationFunctionType.Relu,
                        bias=bi1hd[t * C2:(t + 1) * C2, :], scale=0.5,
                    )
                pso = psmall.tile([P, CW], F32, name="pso", tag="ps0")
                for t in range(2):
                    nc.tensor.matmul(
                        pso[t * C2:(t + 1) * C2, :],
                        ssum[:, t * C2:(t + 1) * C2],
                        xb2[:],
                        start=True, stop=True,
                        tile_position=(0, t * C2),
                    )
                oT = xbpool.tile([P, CW], F32, name="oT")
                nc.vector.tensor_copy(oT[:], pso[:])
                for t in range(2):
                    ck = 2 * pair + t
                    nc.scalar.dma_start(
                        out_e[:, ck * CW:(ck + 1) * CW],
                        oT[t * C2:t * C2 + C, :],
                    )
                ctx_p.__exit__(None, None, None)

            # ================= emission =================
            # pass-1 half 0 (+ Y0 interleaved)
            _mark(nc, "p1h0_start")
            p1acc[0] = pacc.tile([P, CW], F32, name="p1a", tag="acc")
            for ktb in range(NBLK):
                if ktb < 16:
                    y0_block(ktb)
                ft = p1_dma_block(0, ktb)
                p1_mms(0, ktb, ft)
            _mark(nc, "p1h0_end")
            p1_epilogue(0)
            _mark(nc, "epi0_end")

            # pass-1 half 1; yg0 load + p2 acc init issued mid-half so
            # phase A can start the moment half-1's matmuls finish
            import os as _os
            ilv = _os.environ.get("ARMA_ILV") == "1"
            jilv = 3 if ilv else 0
            p1acc[1] = pacc.tile([P, CW], F32, name="p1b", tag="acc")
            for ktb in range(NBLK):
                ft = p1_dma_block(1, ktb)
                p1_mms(1, ktb, ft)
                if ktb == 8:
                    issue_yg(0)
                if ktb == 12:
                    p2_init()
                if ilv and ktb >= 16 and ktb % 4 == 0 and (ktb - 16) // 4 < 3:
                    j = (ktb - 16) // 4
                    for i in range(8):
                        ktb2 = j * 4 + i // 4
                        p2_tile(0, j, i, kept[(0, ktb2)], kept[(1, ktb2)])
            _mark(nc, "p1h1_end")
            p1_epilogue(1)
            _mark(nc, "epi1_end")
            issue_yg(1)

            # pinned phase A (j = jilv..JPIN-1)
            for j in range(jilv, JPIN):
                for i in range(8):
                    ktb2 = j * 4 + i // 4
                    p2_tile(0, j, i, kept[(0, ktb2)], kept[(1, ktb2)])
            _mark(nc, "phA_pinned_end")
            # streamed phase A (j = JPIN..7)
            for j in range(JPIN, NCORES):
                for blk_i in range(2):
                    fts = p2_stream_block(0, j, blk_i)
                    for i in range(4 * blk_i, 4 * blk_i + 4):
                        p2_tile(0, j, i, fts[0], fts[1])
            _mark(nc, "phA_end")
            # phase B (gather half 1); last block split for epilogue hiding
            for j in range(NCORES):
                for blk_i in range(2):
                    lastblk = j == NCORES - 1 and blk_i == 1
                    fts = p2_stream_block(1, j, blk_i)
                    if not lastblk:
                        for i in range(4 * blk_i, 4 * blk_i + 4):
                            p2_tile(1, j, i, fts[0], fts[1])
                    else:
                        for i in range(4, 8):
                            # chunks 0,1 first; stop their groups
                            b = i % 4
                            yt = ygt[2 + j // 4][
                                :, ((j % 4) * 8 + i) * C2:
                                ((j % 4) * 8 + i + 1) * C2]
                            nc.tensor.matmul(
                                p2acc[0][0:C2, :], yt, fts[0][:, b, 0:CW],
                                start=False, stop=(i == 7),
                                tile_position=(0, 0))
                            nc.tensor.matmul(
                                p2acc[0][C2:P, :], yt, fts[0][:, b, CW:HW_],
                                start=False, stop=(i == 7),
                                tile_position=(0, C2))
                        p2_epilogue(0)
                        for i in range(4, 8):
                            b = i % 4
                            yt = ygt[2 + j // 4][
                                :, ((j % 4) * 8 + i) * C2:
                                ((j % 4) * 8 + i + 1) * C2]
                            nc.tensor.matmul(
                                p2acc[1][0:C2, :], yt, fts[1][:, b, 0:CW],
                                start=False, stop=(i == 7),
                                tile_position=(0, 0))
                            nc.tensor.matmul(
                                p2acc[1][C2:P, :], yt, fts[1][:, b, CW:HW_],
                                start=False, stop=(i == 7),
                                tile_position=(0, C2))
                        p2_epilogue(1)

    nc.compile()
    return nc


def kernel(**inputs):
    x = np.ascontiguousarray(np.asarray(inputs["x"], dtype=np.float32))
    fltr = np.ascontiguousarray(np.asarray(inputs["fltr"], dtype=np.float32))

    def cat(a, b, axis=1):
        return np.ascontiguousarray(
            np.concatenate(
                [np.asarray(a, np.float32), np.asarray(b, np.float32)],
                axis=axis,
            )
        )

    f8 = ml_dtypes.float8_e3m4
    bf = ml_dtypes.bfloat16

    # fused conv kernels, descaled by 2^-8 (fp8 fold)
    w1i0f = (cat(inputs["k0i0_w1"], inputs["k1i0_w1"]) / FSCALE)  # [64,64]
    w1i1f = np.zeros((C2, C2), dtype=np.float32)
    w1i1f[0:C, 0:C] = np.asarray(inputs["k0i1_w1"], np.float32)
    w1i1f[C:C2, C:C2] = np.asarray(inputs["k1i1_w1"], np.float32)
    w1i1f = w1i1f / FSCALE

    def dpad(w):  # [[w,0],[0,w]] -> [128, 128]
        o = np.zeros((P, P), dtype=np.float32)
        o[0:C2, 0:C2] = w
        o[C2:P, C2:P] = w
        return o

    w1i0p = np.ascontiguousarray(dpad(w1i0f).astype(bf))
    w1i1p = np.ascontiguousarray(dpad(w1i1f).astype(bf))

    def zpad(w):  # [w; 0] -> [128, 64]
        o = np.zeros((P, C2), dtype=np.float32)
        o[0:C2, :] = w
        return np.ascontiguousarray(o)

    w2i0p = zpad(cat(inputs["k0i0_w2"], inputs["k1i0_w2"])).astype(bf)
    w2i1p = zpad(cat(inputs["k0i1_w2"], inputs["k1i1_w2"])).astype(bf)
    bi0d = np.ascontiguousarray(
        np.tile(cat(inputs["k0i0_b"], inputs["k1i0_b"], axis=0), 2)[:, None])
    bi1hd = np.ascontiguousarray(
        0.5 * np.tile(cat(inputs["k0i1_b"], inputs["k1i1_b"], axis=0),
                      2)[:, None])

    # stack-mean selection: out[m] = xb2[m] + xb2[m+32] for m < 32
    ssum = np.zeros((P, P), dtype=np.float32)
    for m in range(C):
        ssum[m, m] = ssum[m + C, m] = 1.0
        ssum[C2 + m, C2 + m] = ssum[C2 + C + m, C2 + m] = 1.0
    ssum = np.ascontiguousarray(ssum.astype(bf))

    xT = x.T.astype(bf)                      # [64, 16384]
    xt = np.ascontiguousarray(
        np.concatenate([xT[:, 0:N // 2], xT[:, N // 2:N]], axis=0))

    # fp8 E3M4 fltr at rest, transposed per core, scaled by 2^8
    fltrs = (fltr * np.float32(FSCALE)).astype(f8)

    if "nc" not in _CACHE:
        _CACHE["nc"] = _build()
    nc = _CACHE["nc"]

    in_maps = []
    for m in range(NCORES):
        rows = slice(m * R, (m + 1) * R)
        xtm = np.zeros((P, R), dtype=np.float32)
        xtm[0:F, :] = x[rows, :].T
        xtm = xtm.astype(bf)
        in_maps.append({
            "fltrt0": np.ascontiguousarray(fltrs[m * R:m * R + HW_, :].T),
            "fltrt1": np.ascontiguousarray(
                fltrs[m * R + HW_:(m + 1) * R, :].T),
            "xt": xt,
            "xtm": np.ascontiguousarray(xtm),
            "w1i0p": w1i0p, "w1i1p": w1i1p,
            "w2i0p": w2i0p, "w2i1p": w2i1p,
            "bi0d": bi0d, "bi1hd": bi1hd, "ssum": ssum,
        })

    import os
    import time
    trace = os.environ.get("ARMA_TRACE") == "1"
    last_exc = None
    for attempt in range(3):
        try:
            res = run_bass_kernel_spmd(
                nc, in_maps, core_ids=list(range(NCORES)), trace=trace,
            )
            break
        except Exception as e:  # transient NRT device errors: retry
            last_exc = e
            time.sleep(5.0)
    else:
        raise last_exc
    _CACHE["last_results"] = res
    out = np.concatenate(
        [np.asarray(res.results[m]["out"]).T for m in range(NCORES)], axis=0
    )
    return out


# revision 15
# speedup vs baseline: 1.2475x; 1.0543x over previous
"""Distributed ARMAConv kernel for 8 TRN2 NeuronCores (Bass/Tile).

Reference computation (N=16384 nodes, F=64 in-feats, C=32 channels,
K=2 stacks, T=2 iterations):
    for each stack k:  xbar = x
        for i in 0..1: xbar = relu(fltr @ (xbar @ w1) + x @ w2 + b)
    out = mean over stacks                                  -> [N, 32]

Strategy (v2 - 2x column-tiled PE):
  - Row-shard fltr across 8 cores; core m holds fltr[rows_m, :] stored
    TRANSPOSED as two contiguous half-arrays (1 KiB DMA lines), fp8
    E3M4 at rest, pre-scaled by 2^8 (descale folded into w1).
  - Fuse the two ARMA stacks: Y = [xbar@w1_k0 | xbar@w1_k1] is [N,64],
    so fltr streams once per iteration.
  - The stationary operand (Y tile [128,64]) only fills half the
    128-wide PE array.  ALL matmuls run 2x column-tiled (tile_size
    (128,64)): tile (0,0) -> PSUM partitions 0-63, tile (0,64) ->
    64-127, each with its own moving fltr stream.  Measured 519 ns per
    kt-tile (2 concurrent 512-wide fp8 streams) vs 1034 serial -> PE
    ~67us per pass instead of ~110, making the kernel DMA-bound.
  - Every matmul in the kernel keeps tile_size (128,64) (no mode
    switches): K=64 matmuls (Y0 = x@w1, Y1 = relu@w1i1, w2-terms,
    final stack-mean) are zero-padded to K=128, with zeros placed in
    the operand that multiplies the junk rows.  The final stack-mean
    (out = 0.5*(relu_lo + relu_hi)) runs on the PE via a 0/1
    selection stationary instead of partition-shift DMA + DVE add.
  - The x@w2 bias term is accumulated LAST (stop) instead of first,
    so xm's DMA is off the critical path; the first fltr matmul
    carries start=True.
  - SBUF pinning: the pass-1 fltr blocks that pass-2 phase A (gather
    half 0, low core-blocks) consumes are kept resident (JPIN blocks
    per half-array stream); phase-A matmuls for those blocks are
    interleaved into pass-1 half-1's DMA-bound stream, filling PE
    idle slots.  Pass 2 re-reads only the rest.
  - Big fltr DMAs ride the sync-engine HWDGE ring; small/latency DMAs
    ride the scalar-engine ring; collectives keep the gpsimd queue.
    A dummy warm-up collective at t=0 absorbs the one-time rendezvous
    barrier (~60us) that would otherwise delay gather 0.
"""

import numpy as np
import ml_dtypes

import concourse.mybir as mybir
import concourse.tile as tile
from concourse import bacc
from concourse.bass_utils import run_bass_kernel_spmd

N = 16384            # nodes
F = 64               # input features
C = 32               # channels per stack
C2 = 2 * C           # fused channels (2 stacks)
NCORES = 8
R = N // NCORES      # fltr rows per core (2048)
P = 128              # partitions
NKT = N // P         # kt tiles per full pass (128)
HW_ = R // 2         # 1024 output rows per half-array
CW = 512             # output rows per chunk / PSUM accumulator slice
KB1 = 4              # kt tiles per fltr DMA block (512 KiB)
NBLK = NKT // KB1    # 32 blocks per half-array
JPIN = 6             # pin blocks {4j,4j+1: j<JPIN} of both half-arrays
FSCALE = 256.0       # power-of-2 fp8 pre-scale (folded into w1)

F32 = mybir.dt.float32
F32R = mybir.dt.float32r
BF16 = mybir.dt.bfloat16
F8 = mybir.dt.float8e3

_CACHE = {}


PHASES = []


def _mark(nc, label):
    PHASES.append((label, sum(1 for _ in nc.all_instructions())))


def _build():
    nc = bacc.Bacc(
        trn_type="TRN2", target_bir_lowering=False, debug=False,
        num_devices=NCORES,
    )
    # block-interleaved fltr: row ktb*128+p holds that partition's 4
    # kt-rows contiguously -> 4 KiB DMA descriptors (near-ceiling HBM rate)
    fltrT0_e = nc.dram_tensor("fltrt0", [NBLK * P, KB1 * HW_], F8,
                              kind="ExternalInput")
    fltrT1_e = nc.dram_tensor("fltrt1", [NBLK * P, KB1 * HW_], F8,
                              kind="ExternalInput")
    xt_e = nc.dram_tensor("xt", [P, N // 2], BF16, kind="ExternalInput")
    xtm_e = nc.dram_tensor("xtm", [P, R], BF16, kind="ExternalInput")
    w1i0_e = nc.dram_tensor("w1i0p", [P, P], BF16, kind="ExternalInput")
    w1i1_e = nc.dram_tensor("w1i1p", [P, P], BF16, kind="ExternalInput")
    w2i0_e = nc.dram_tensor("w2i0p", [P, C2], BF16, kind="ExternalInput")
    w2i1_e = nc.dram_tensor("w2i1p", [P, C2], BF16, kind="ExternalInput")
    bi0_e = nc.dram_tensor("bi0d", [P, 1], F32, kind="ExternalInput")
    bi1h_e = nc.dram_tensor("bi1hd", [P, 1], F32, kind="ExternalInput")
    ssum_e = nc.dram_tensor("ssum", [P, P], BF16, kind="ExternalInput")
    out_e = nc.dram_tensor("out", [C, R], F32, kind="ExternalOutput")

    RG = [list(range(NCORES))]
    fltr_halves = [fltrT0_e, fltrT1_e]

    with tile.TileContext(nc) as tc:
        with (
            tc.tile_pool(name="wpool", bufs=1) as wpool,
            tc.tile_pool(name="y0pool", bufs=1) as y0pool,
            tc.tile_pool(name="xtq", bufs=4) as xtqpool,
            tc.tile_pool(name="kpool", bufs=1) as kpool,
            tc.tile_pool(name="fpool", bufs=8) as fpool,
            tc.tile_pool(name="xbpool", bufs=2) as xbpool,
            tc.tile_pool(name="ylpool", bufs=2) as ylpool,
            tc.tile_pool(name="pacc", bufs=4, space="PSUM") as pacc,
            tc.tile_pool(name="psmall", bufs=2, space="PSUM") as psmall,
            tc.tile_pool(name="dram", bufs=8, space="DRAM") as dram,
        ):
            # ---- resident small tensors (scalar ring) + xt (sync ring,
            # ---- ahead of the fltr stream; 4 independent tiles)
            w1i0p = wpool.tile([P, P], BF16)
            nc.scalar.dma_start(w1i0p[:], w1i0_e[:])
            xts = []
            for q in range(4):
                xq = xtqpool.tile([P, N // 8], BF16, name=f"xt{q}",
                                  tag="xtq")
                nc.scalar.dma_start(xq[:],
                                    xt_e[:, q * (N // 8):(q + 1) * (N // 8)])
                xts.append(xq)

            # dummy warm-up collective: anchors the one-time rendezvous
            # barrier while all cores are still in startup.
            gwin = dram.tile([F, C2], BF16, name="gwin", tag="gwin")
            nc.scalar.dma_start(gwin[:], w1i0p[0:F, 0:C2])
            gwout = dram.tile([NCORES * F, C2], BF16, name="gwout",
                              tag="gwout", addr_space="Shared")
            nc.gpsimd.collective_compute(
                "AllGather", mybir.AluOpType.bypass,
                replica_groups=RG,
                ins=[gwin[:].opt()], outs=[gwout[:].opt()],
            )

            w1i1p = wpool.tile([P, P], BF16)
            nc.scalar.dma_start(w1i1p[:], w1i1_e[:])
            w2i0p = wpool.tile([P, C2], BF16)
            nc.scalar.dma_start(w2i0p[:], w2i0_e[:])
            w2i1p = wpool.tile([P, C2], BF16)
            nc.scalar.dma_start(w2i1p[:], w2i1_e[:])
            bi0d = wpool.tile([P, 1], F32)
            nc.scalar.dma_start(bi0d[:], bi0_e[:])
            bi1hd = wpool.tile([P, 1], F32)
            nc.scalar.dma_start(bi1hd[:], bi1h_e[:])
            ssum = wpool.tile([P, P], BF16)
            nc.scalar.dma_start(ssum[:], ssum_e[:])
            xm = wpool.tile([P, R], BF16)
            nc.scalar.dma_start(xm[:], xtm_e[:])

            y0 = y0pool.tile([P, NKT * C2], BF16, tag="y0")

            def y0_block(b):
                # twin-node: one N=128 matmul computes Y0 for a 64-node
                # lo-half group (xt rows 0-63 x w1i0p cols 0:64) AND its
                # hi-half twin (rows 64-127 x cols 64:128).  Block b
                # covers lo nodes [512b, 512b+512) = lo kt 4b..4b+4 and
                # their twins kt 64+4b..; psum placement makes the
                # evacuation one contiguous [128, 512] copy.
                ps = psmall.tile([P, 4, P], F32, name="ps0", tag="ps0")
                q4 = b // 4
                base = b * 512 - q4 * (N // 8)
                xq = xts[q4]
                for q in range(8):
                    col = base + q * C2
                    nc.tensor.matmul(
                        ps[(q % 2) * C2:(q % 2) * C2 + C2, q // 2, :],
                        xq[:, col:col + C2], w1i0p[:],
                        start=True, stop=True,
                        tile_position=(0, (q % 2) * C2),
                    )
                nc.vector.tensor_copy(
                    y0[:, b * CW:(b + 1) * CW],
                    ps[:].rearrange("p u c -> p (u c)"))

            # ---- pass-1 state
            p1acc = [None, None]
            kept = {}

            def p1_dma_block(h, ktb):
                if ktb % 4 < 2 and ktb // 4 < JPIN:
                    ft = kpool.tile([P, KB1, HW_], F8, name="ftk",
                                    tag="ftk", bufs=4 * JPIN)
                    kept[(h, ktb)] = ft
                else:
                    ft = fpool.tile([P, KB1, HW_], F8, name="ft", tag="ft")
                nc.sync.dma_start(
                    ft[:],
                    fltr_halves[h][ktb * P:(ktb + 1) * P, :]
                    .rearrange("p (b c) -> p b c", c=HW_),
                )
                return ft

            def y0sl(kt):
                if kt < NKT // 2:
                    return y0[:, kt * P:kt * P + C2]
                return y0[:, (kt - NKT // 2) * P + C2:(kt - NKT // 2) * P + P]

            def p1_mms(h, ktb, ft):
                acc = p1acc[h]
                for b in range(KB1):
                    kt = ktb * KB1 + b
                    first = kt == 0
                    yt = y0sl(kt)
                    nc.tensor.matmul(acc[0:C2, :], yt, ft[:, b, 0:CW],
                                     start=first, stop=False,
                                     tile_position=(0, 0))
                    nc.tensor.matmul(acc[C2:P, :], yt, ft[:, b, CW:HW_],
                                     start=first, stop=False,
                                     tile_position=(0, C2))

            def p1_epilogue(h):
                ctx_p = tc.high_priority(offset=600)
                ctx_p.__enter__()
                acc = p1acc[h]
                # x@w2 term, contracted over zero-padded K=128 (stop)
                nc.tensor.matmul(acc[0:C2, :], w2i0p[:],
                                 xm[:, 2 * h * CW:(2 * h + 1) * CW],
                                 start=False, stop=True,
                                 tile_position=(0, 0))
                nc.tensor.matmul(acc[C2:P, :], w2i0p[:],
                                 xm[:, (2 * h + 1) * CW:(2 * h + 2) * CW],
                                 start=False, stop=True,
                                 tile_position=(0, C2))
                xb1 = xbpool.tile([P, CW], BF16, name="xb1")
                nc.scalar.activation(
                    xb1[0:C2, :], acc[0:C2, :],
                    mybir.ActivationFunctionType.Relu,
                    bias=bi0d[0:C2, :], scale=1.0,
                )
                nc.scalar.activation(
                    xb1[C2:P, :], acc[C2:P, :],
                    mybir.ActivationFunctionType.Relu,
                    bias=bi0d[C2:P, :], scale=1.0,
                )
                psy = psmall.tile([P, 4, P], F32, name="psy", tag="ps0")
                for g in range(8):
                    nc.tensor.matmul(
                        psy[(g % 2) * C2:(g % 2) * C2 + C2, g // 2, :],
                        xb1[:, g * C2:(g + 1) * C2], w1i1p[:],
                        start=True, stop=True,
                        tile_position=(0, (g % 2) * C2),
                    )
                y1h = ylpool.tile([P, 8, C2], BF16, name="y1h")
                nc.vector.tensor_copy(y1h[:, 0:4, :], psy[:, :, 0:C2])
                nc.vector.tensor_copy(y1h[:, 4:8, :], psy[:, :, C2:P])
                # p-major gather payload: per-partition-contiguous 1 KiB
                gin = dram.tile([P, 8 * C2], BF16, name="gin", tag="gin",
                                bufs=2)
                nc.scalar.dma_start(
                    gin[:], y1h[:].rearrange("p t ch -> p (t ch)"))
                gout = dram.tile(
                    [NCORES * P, 8 * C2], BF16, name="gout", tag="gout",
                    addr_space="Shared", bufs=2,
                )
                nc.gpsimd.collective_compute(
                    "AllGather", mybir.AluOpType.bypass,
                    replica_groups=RG,
                    ins=[gin[:].opt()], outs=[gout[:].opt()],
                )
                gouts.append(gout)
                ctx_p.__exit__(None, None, None)

            gouts = []
            ygt = [None] * 4
            yg_issued = [False] * 2

            def issue_yg(hg):
                if yg_issued[hg]:
                    return
                with tc.high_priority(offset=600):
                    for hh in range(2):
                        t = xtqpool.tile([P, N // 8], BF16,
                                         name=f"yg{hg}{hh}", tag="xtq")
                        ygt[hg * 2 + hh] = t
                        nc.scalar.dma_start(
                            t[:].rearrange("p (m c) -> p m c", c=8 * C2),
                            gouts[hg][hh * 4 * P:(hh * 4 + 4) * P, :]
                            .rearrange("(m p) c -> p m c", p=P),
                        )
                yg_issued[hg] = True

            # ---- pass-2 state
            p2acc = [None, None]   # [chunks 0|1, chunks 2|3]

            def p2_init():
                p2acc[0] = pacc.tile([P, CW], F32, name="p2a", tag="acc")
                p2acc[1] = pacc.tile([P, CW], F32, name="p2b", tag="acc")
                for pair in range(2):
                    for t in range(2):
                        ck = 2 * pair + t
                        nc.tensor.matmul(
                            p2acc[pair][t * C2:(t + 1) * C2, :],
                            w2i1p[:], xm[:, ck * CW:(ck + 1) * CW],
                            start=True, stop=False,
                            tile_position=(0, t * C2),
                        )

            def p2_tile(hg, j, i, ft0, ft1, stop=False):
                b = i % 4
                yt = ygt[hg * 2 + j // 4][
                    :, ((j % 4) * 8 + i) * C2:((j % 4) * 8 + i + 1) * C2]
                nc.tensor.matmul(p2acc[0][0:C2, :], yt, ft0[:, b, 0:CW],
                                 start=False, stop=stop,
                                 tile_position=(0, 0))
                nc.tensor.matmul(p2acc[0][C2:P, :], yt, ft0[:, b, CW:HW_],
                                 start=False, stop=stop,
                                 tile_position=(0, C2))
                if ft1 is None:
                    return
                nc.tensor.matmul(p2acc[1][0:C2, :], yt, ft1[:, b, 0:CW],
                                 start=False, stop=stop,
                                 tile_position=(0, 0))
                nc.tensor.matmul(p2acc[1][C2:P, :], yt, ft1[:, b, CW:HW_],
                                 start=False, stop=stop,
                                 tile_position=(0, C2))

            def p2_stream_block(hg, j, blk_i):
                # DMA half-array 0/1 blocks for (j, i in [4*blk_i,+4))
                ktb = j * 4 + 2 * hg + blk_i
                fts = []
                for ha in range(2):
                    ft = kept.get((ha, ktb))
                    if ft is None:
                        ft = fpool.tile([P, KB1, HW_], F8, name="ft2",
                                        tag="ft")
                        nc.sync.dma_start(
                            ft[:],
                            fltr_halves[ha][ktb * P:(ktb + 1) * P, :]
                            .rearrange("p (b c) -> p b c", c=HW_),
                        )
                    fts.append(ft)
                return fts

            def p2_epilogue(pair):
                ctx_p = tc.high_priority(offset=600)
                ctx_p.__enter__()
                acc = p2acc[pair]
                xb2 = xbpool.tile([P, CW], BF16, name="xb2")
                for t in range(2):
                    nc.scalar.activation(
                        xb2[t * C2:(t + 1) * C2, :],
                        acc[t * C2:(t + 1) * C2, :],
                        mybir.ActivationFunctionType.Relu,
                        bias=bi1hd[t * C2:(t + 1) * C2, :], scale=0.5,
                    )
                pso = psmall.tile([P, CW], F32, name="pso", tag="ps0")
                for t in range(2):
                    nc.tensor.matmul(
                        pso[t * C2:(t + 1) * C2, :],
                        ssum[:, t * C2:(t + 1) * C2],
                        xb2[:],
                        start=True, stop=True,
                        tile_position=(0, t * C2),
                    )
                oT = xbpool.tile([P, CW], F32, name="oT")
                nc.vector.tensor_copy(oT[:], pso[:])
                for t in range(2):
                    ck = 2 * pair + t
                    nc.scalar.dma_start(
                        out_e[:, ck * CW:(ck + 1) * CW],
                        oT[t * C2:t * C2 + C, :],
                    )
                ctx_p.__exit__(None, None, None)

            # ================= emission =================
            # pass-1 half 0 (+ Y0 interleaved)
            _mark(nc, "p1h0_start")
            p1acc[0] = pacc.tile([P, CW], F32, name="p1a", tag="acc")
            for ktb in range(NBLK):
                if ktb < 16:
                    y0_block(ktb)
                ft = p1_dma_block(0, ktb)
                p1_mms(0, ktb, ft)
            _mark(nc, "p1h0_end")
            p1_epilogue(0)
            _mark(nc, "epi0_end")

            # pass-1 half 1; yg0 load + p2 acc init issued mid-half so
            # phase A can start the moment half-1's matmuls finish
            import os as _os
            ilv = _os.environ.get("ARMA_ILV") == "1"
            jilv = 3 if ilv else 0
            p1acc[1] = pacc.tile([P, CW], F32, name="p1b", tag="acc")
            for ktb in range(NBLK):
                ft = p1_dma_block(1, ktb)
                p1_mms(1, ktb, ft)
                if ktb == 8:
                    issue_yg(0)
                if ktb == 12:
                    p2_init()
                if ilv and ktb >= 16 and ktb % 4 == 0 and (ktb - 16) // 4 < 3:
                    j = (ktb - 16) // 4
                    for i in range(8):
                        ktb2 = j * 4 + i // 4
                        p2_tile(0, j, i, kept[(0, ktb2)], kept[(1, ktb2)])
            _mark(nc, "p1h1_end")
            p1_epilogue(1)
            _mark(nc, "epi1_end")
            issue_yg(1)

            # pinned phase A (j = jilv..JPIN-1)
            for j in range(jilv, JPIN):
                for i in range(8):
                    ktb2 = j * 4 + i // 4
                    p2_tile(0, j, i, kept[(0, ktb2)], kept[(1, ktb2)])
            _mark(nc, "phA_pinned_end")
            # streamed phase A (j = JPIN..7)
            for j in range(JPIN, NCORES):
                for blk_i in range(2):
                    fts = p2_stream_block(0, j, blk_i)
                    for i in range(4 * blk_i, 4 * blk_i + 4):
                        p2_tile(0, j, i, fts[0], fts[1])
            _mark(nc, "phA_end")
            # phase B (gather half 1); last block split for epilogue hiding
            for j in range(NCORES):
                for blk_i in range(2):
                    lastblk = j == NCORES - 1 and blk_i == 1
                    fts = p2_stream_block(1, j, blk_i)
                    if not lastblk:
                        for i in range(4 * blk_i, 4 * blk_i + 4):
                            p2_tile(1, j, i, fts[0], fts[1])
                    else:
                        for i in range(4, 8):
                            # chunks 0,1 first; stop their groups
                            b = i % 4
                            yt = ygt[2 + j // 4][
                                :, ((j % 4) * 8 + i) * C2:
                                ((j % 4) * 8 + i + 1) * C2]
                            nc.tensor.matmul(
                                p2acc[0][0:C2, :], yt, fts[0][:, b, 0:CW],
                                start=False, stop=(i == 7),
                                tile_position=(0, 0))
                            nc.tensor.matmul(
                                p2acc[0][C2:P, :], yt, fts[0][:, b, CW:HW_],
                                start=False, stop=(i == 7),
                                tile_position=(0, C2))
                        p2_epilogue(0)
                        for i in range(4, 8):
                            b = i % 4
                            yt = ygt[2 + j // 4][
                                :, ((j % 4) * 8 + i) * C2:
                                ((j % 4) * 8 + i + 1) * C2]
                            nc.tensor.matmul(
                                p2acc[1][0:C2, :], yt, fts[1][:, b, 0:CW],
                                start=False, stop=(i == 7),
                                tile_position=(0, 0))
                            nc.tensor.matmul(
                                p2acc[1][C2:P, :], yt, fts[1][:, b, CW:HW_],
                                start=False, stop=(i == 7),
                                tile_position=(0, C2))
                        p2_epilogue(1)

    nc.compile()
    return nc


def kernel(**inputs):
    x = np.ascontiguousarray(np.asarray(inputs["x"], dtype=np.float32))
    fltr = np.ascontiguousarray(np.asarray(inputs["fltr"], dtype=np.float32))

    def cat(a, b, axis=1):
        return np.ascontiguousarray(
            np.concatenate(
                [np.asarray(a, np.float32), np.asarray(b, np.float32)],
                axis=axis,
            )
        )

    f8 = ml_dtypes.float8_e3m4
    bf = ml_dtypes.bfloat16

    # fused conv kernels, descaled by 2^-8 (fp8 fold)
    w1i0f = (cat(inputs["k0i0_w1"], inputs["k1i0_w1"]) / FSCALE)  # [64,64]
    w1i1f = np.zeros((C2, C2), dtype=np.float32)
    w1i1f[0:C, 0:C] = np.asarray(inputs["k0i1_w1"], np.float32)
    w1i1f[C:C2, C:C2] = np.asarray(inputs["k1i1_w1"], np.float32)
    w1i1f = w1i1f / FSCALE

    def dpad(w):  # [[w,0],[0,w]] -> [128, 128]
        o = np.zeros((P, P), dtype=np.float32)
        o[0:C2, 0:C2] = w
        o[C2:P, C2:P] = w
        return o

    w1i0p = np.ascontiguousarray(dpad(w1i0f).astype(bf))
    w1i1p = np.ascontiguousarray(dpad(w1i1f).astype(bf))

    def zpad(w):  # [w; 0] -> [128, 64]
        o = np.zeros((P, C2), dtype=np.float32)
        o[0:C2, :] = w
        return np.ascontiguousarray(o)

    w2i0p = zpad(cat(inputs["k0i0_w2"], inputs["k1i0_w2"])).astype(bf)
    w2i1p = zpad(cat(inputs["k0i1_w2"], inputs["k1i1_w2"])).astype(bf)
    bi0d = np.ascontiguousarray(
        np.tile(cat(inputs["k0i0_b"], inputs["k1i0_b"], axis=0), 2)[:, None])
    bi1hd = np.ascontiguousarray(
        0.5 * np.tile(cat(inputs["k0i1_b"], inputs["k1i1_b"], axis=0),
                      2)[:, None])

    # stack-mean selection: out[m] = xb2[m] + xb2[m+32] for m < 32
    ssum = np.zeros((P, P), dtype=np.float32)
    for m in range(C):
        ssum[m, m] = ssum[m + C, m] = 1.0
        ssum[C2 + m, C2 + m] = ssum[C2 + C + m, C2 + m] = 1.0
    ssum = np.ascontiguousarray(ssum.astype(bf))

    xT = x.T.astype(bf)                      # [64, 16384]
    xt = np.ascontiguousarray(
        np.concatenate([xT[:, 0:N // 2], xT[:, N // 2:N]], axis=0))

    # fp8 E3M4 fltr at rest, transposed per core, scaled by 2^8
    fltrs = (fltr * np.float32(FSCALE)).astype(f8)

    if "nc" not in _CACHE:
        _CACHE["nc"] = _build()
    nc = _CACHE["nc"]

    in_maps = []
    for m in range(NCORES):
        rows = slice(m * R, (m + 1) * R)
        xtm = np.zeros((P, R), dtype=np.float32)
        xtm[0:F, :] = x[rows, :].T
        xtm = xtm.astype(bf)
        def blk(a):  # [N, HW_] -> [NBLK*P, KB1*HW_] block-interleaved
            return np.ascontiguousarray(
                a.reshape(NBLK, KB1, P, HW_).transpose(0, 2, 1, 3)
                .reshape(NBLK * P, KB1 * HW_))

        in_maps.append({
            "fltrt0": blk(np.ascontiguousarray(
                fltrs[m * R:m * R + HW_, :].T)),
            "fltrt1": blk(np.ascontiguousarray(
                fltrs[m * R + HW_:(m + 1) * R, :].T)),
            "xt": xt,
            "xtm": np.ascontiguousarray(xtm),
            "w1i0p": w1i0p, "w1i1p": w1i1p,
            "w2i0p": w2i0p, "w2i1p": w2i1p,
            "bi0d": bi0d, "bi1hd": bi1hd, "ssum": ssum,
        })

    import os
    import time
    trace = os.environ.get("ARMA_TRACE") == "1"
    last_exc = None
    for attempt in range(3):
        try:
            res = run_bass_kernel_spmd(
                nc, in_maps, core_ids=list(range(NCORES)), trace=trace,
            )
            break
        except Exception as e:  # transient NRT device errors: retry
            last_exc = e
            time.sleep(5.0)
    else:
        raise last_exc
    _CACHE["last_results"] = res
    out = np.concatenate(
        [np.asarray(res.results[m]["out"]).T for m in range(NCORES)], axis=0
    )
    return out


# revision 16
# speedup vs baseline: 1.2543x; 1.0054x over previous
"""Distributed ARMAConv kernel for 8 TRN2 NeuronCores (Bass/Tile).

Reference computation (N=16384 nodes, F=64 in-feats, C=32 channels,
K=2 stacks, T=2 iterations):
    for each stack k:  xbar = x
        for i in 0..1: xbar = relu(fltr @ (xbar @ w1) + x @ w2 + b)
    out = mean over stacks                                  -> [N, 32]

Strategy (v2 - 2x column-tiled PE):
  - Row-shard fltr across 8 cores; core m holds fltr[rows_m, :] stored
    TRANSPOSED as two contiguous half-arrays (1 KiB DMA lines), fp8
    E3M4 at rest, pre-scaled by 2^8 (descale folded into w1).
  - Fuse the two ARMA stacks: Y = [xbar@w1_k0 | xbar@w1_k1] is [N,64],
    so fltr streams once per iteration.
  - The stationary operand (Y tile [128,64]) only fills half the
    128-wide PE array.  ALL matmuls run 2x column-tiled (tile_size
    (128,64)): tile (0,0) -> PSUM partitions 0-63, tile (0,64) ->
    64-127, each with its own moving fltr stream.  Measured 519 ns per
    kt-tile (2 concurrent 512-wide fp8 streams) vs 1034 serial -> PE
    ~67us per pass instead of ~110, making the kernel DMA-bound.
  - Every matmul in the kernel keeps tile_size (128,64) (no mode
    switches): K=64 matmuls (Y0 = x@w1, Y1 = relu@w1i1, w2-terms,
    final stack-mean) are zero-padded to K=128, with zeros placed in
    the operand that multiplies the junk rows.  The final stack-mean
    (out = 0.5*(relu_lo + relu_hi)) runs on the PE via a 0/1
    selection stationary instead of partition-shift DMA + DVE add.
  - The x@w2 bias term is accumulated LAST (stop) instead of first,
    so xm's DMA is off the critical path; the first fltr matmul
    carries start=True.
  - SBUF pinning: the pass-1 fltr blocks that pass-2 phase A (gather
    half 0, low core-blocks) consumes are kept resident (JPIN blocks
    per half-array stream); phase-A matmuls for those blocks are
    interleaved into pass-1 half-1's DMA-bound stream, filling PE
    idle slots.  Pass 2 re-reads only the rest.
  - Big fltr DMAs ride the sync-engine HWDGE ring; small/latency DMAs
    ride the scalar-engine ring; collectives keep the gpsimd queue.
    A dummy warm-up collective at t=0 absorbs the one-time rendezvous
    barrier (~60us) that would otherwise delay gather 0.
"""

import numpy as np
import ml_dtypes

import concourse.mybir as mybir
import concourse.tile as tile
from concourse import bacc
from concourse.bass_utils import run_bass_kernel_spmd

N = 16384            # nodes
F = 64               # input features
C = 32               # channels per stack
C2 = 2 * C           # fused channels (2 stacks)
NCORES = 8
R = N // NCORES      # fltr rows per core (2048)
P = 128              # partitions
NKT = N // P         # kt tiles per full pass (128)
HW_ = R // 2         # 1024 output rows per half-array
CW = 512             # output rows per chunk / PSUM accumulator slice
KB1 = 4              # kt tiles per fltr DMA block (512 KiB)
NBLK = NKT // KB1    # 32 blocks per half-array
JPIN = 6             # pin blocks {4j,4j+1: j<JPIN} of both half-arrays
FSCALE = 256.0       # power-of-2 fp8 pre-scale (folded into w1)

F32 = mybir.dt.float32
F32R = mybir.dt.float32r
BF16 = mybir.dt.bfloat16
F8 = mybir.dt.float8e3

_CACHE = {}


PHASES = []


def _mark(nc, label):
    PHASES.append((label, sum(1 for _ in nc.all_instructions())))


def _build():
    nc = bacc.Bacc(
        trn_type="TRN2", target_bir_lowering=False, debug=False,
        num_devices=NCORES,
    )
    # block-interleaved fltr: row ktb*128+p holds that partition's 4
    # kt-rows contiguously -> 4 KiB DMA descriptors (near-ceiling HBM rate)
    fltrT0_e = nc.dram_tensor("fltrt0", [NBLK * P, KB1 * HW_], F8,
                              kind="ExternalInput")
    fltrT1_e = nc.dram_tensor("fltrt1", [NBLK * P, KB1 * HW_], F8,
                              kind="ExternalInput")
    xt_e = nc.dram_tensor("xt", [P, N // 2], BF16, kind="ExternalInput")
    xtm_e = nc.dram_tensor("xtm", [P, R], BF16, kind="ExternalInput")
    w1i0_e = nc.dram_tensor("w1i0p", [P, P], BF16, kind="ExternalInput")
    w1i1_e = nc.dram_tensor("w1i1p", [P, P], BF16, kind="ExternalInput")
    w2i0_e = nc.dram_tensor("w2i0p", [P, C2], BF16, kind="ExternalInput")
    w2i1_e = nc.dram_tensor("w2i1p", [P, C2], BF16, kind="ExternalInput")
    bi0_e = nc.dram_tensor("bi0d", [P, 1], F32, kind="ExternalInput")
    bi1h_e = nc.dram_tensor("bi1hd", [P, 1], F32, kind="ExternalInput")
    ssum_e = nc.dram_tensor("ssum", [P, P], BF16, kind="ExternalInput")
    out_e = nc.dram_tensor("out", [C, R], F32, kind="ExternalOutput")

    RG = [list(range(NCORES))]
    fltr_halves = [fltrT0_e, fltrT1_e]

    with tile.TileContext(nc) as tc:
        with (
            tc.tile_pool(name="wpool", bufs=1) as wpool,
            tc.tile_pool(name="y0pool", bufs=1) as y0pool,
            tc.tile_pool(name="xtq", bufs=4) as xtqpool,
            tc.tile_pool(name="kpool", bufs=1) as kpool,
            tc.tile_pool(name="fpool", bufs=10) as fpool,
            tc.tile_pool(name="xbpool", bufs=2) as xbpool,
            tc.tile_pool(name="ylpool", bufs=2) as ylpool,
            tc.tile_pool(name="pacc", bufs=4, space="PSUM") as pacc,
            tc.tile_pool(name="psmall", bufs=2, space="PSUM") as psmall,
            tc.tile_pool(name="dram", bufs=8, space="DRAM") as dram,
        ):
            # ---- resident small tensors (scalar ring) + xt (sync ring,
            # ---- ahead of the fltr stream; 4 independent tiles)
            w1i0p = wpool.tile([P, P], BF16)
            nc.scalar.dma_start(w1i0p[:], w1i0_e[:])
            xts = []
            for q in range(4):
                xq = xtqpool.tile([P, N // 8], BF16, name=f"xt{q}",
                                  tag="xtq")
                nc.scalar.dma_start(xq[:],
                                    xt_e[:, q * (N // 8):(q + 1) * (N // 8)])
                xts.append(xq)

            # dummy warm-up collective: anchors the one-time rendezvous
            # barrier while all cores are still in startup.
            gwin = dram.tile([F, C2], BF16, name="gwin", tag="gwin")
            nc.scalar.dma_start(gwin[:], w1i0p[0:F, 0:C2])
            gwout = dram.tile([NCORES * F, C2], BF16, name="gwout",
                              tag="gwout", addr_space="Shared")
            nc.gpsimd.collective_compute(
                "AllGather", mybir.AluOpType.bypass,
                replica_groups=RG,
                ins=[gwin[:].opt()], outs=[gwout[:].opt()],
            )

            w1i1p = wpool.tile([P, P], BF16)
            nc.scalar.dma_start(w1i1p[:], w1i1_e[:])
            w2i0p = wpool.tile([P, C2], BF16)
            nc.scalar.dma_start(w2i0p[:], w2i0_e[:])
            w2i1p = wpool.tile([P, C2], BF16)
            nc.scalar.dma_start(w2i1p[:], w2i1_e[:])
            bi0d = wpool.tile([P, 1], F32)
            nc.scalar.dma_start(bi0d[:], bi0_e[:])
            bi1hd = wpool.tile([P, 1], F32)
            nc.scalar.dma_start(bi1hd[:], bi1h_e[:])
            ssum = wpool.tile([P, P], BF16)
            nc.scalar.dma_start(ssum[:], ssum_e[:])
            xm = wpool.tile([P, R], BF16)
            nc.scalar.dma_start(xm[:], xtm_e[:])

            y0 = y0pool.tile([P, NKT * C2], BF16, tag="y0")

            def y0_block(b):
                # twin-node: one N=128 matmul computes Y0 for a 64-node
                # lo-half group (xt rows 0-63 x w1i0p cols 0:64) AND its
                # hi-half twin (rows 64-127 x cols 64:128).  Block b
                # covers lo nodes [512b, 512b+512) = lo kt 4b..4b+4 and
                # their twins kt 64+4b..; psum placement makes the
                # evacuation one contiguous [128, 512] copy.
                ps = psmall.tile([P, 4, P], F32, name="ps0", tag="ps0")
                q4 = b // 4
                base = b * 512 - q4 * (N // 8)
                xq = xts[q4]
                for q in range(8):
                    col = base + q * C2
                    nc.tensor.matmul(
                        ps[(q % 2) * C2:(q % 2) * C2 + C2, q // 2, :],
                        xq[:, col:col + C2], w1i0p[:],
                        start=True, stop=True,
                        tile_position=(0, (q % 2) * C2),
                    )
                nc.vector.tensor_copy(
                    y0[:, b * CW:(b + 1) * CW],
                    ps[:].rearrange("p u c -> p (u c)"))

            # ---- pass-1 state
            p1acc = [None, None]
            kept = {}

            def p1_dma_block(h, ktb):
                if ktb % 4 < 2 and ktb // 4 < JPIN:
                    ft = kpool.tile([P, KB1, HW_], F8, name="ftk",
                                    tag="ftk", bufs=4 * JPIN)
                    kept[(h, ktb)] = ft
                else:
                    ft = fpool.tile([P, KB1, HW_], F8, name="ft", tag="ft")
                nc.sync.dma_start(
                    ft[:],
                    fltr_halves[h][ktb * P:(ktb + 1) * P, :]
                    .rearrange("p (b c) -> p b c", c=HW_),
                )
                return ft

            def y0sl(kt):
                if kt < NKT // 2:
                    return y0[:, kt * P:kt * P + C2]
                return y0[:, (kt - NKT // 2) * P + C2:(kt - NKT // 2) * P + P]

            def p1_mms(h, ktb, ft):
                acc = p1acc[h]
                for b in range(KB1):
                    kt = ktb * KB1 + b
                    first = kt == 0
                    yt = y0sl(kt)
                    nc.tensor.matmul(acc[0:C2, :], yt, ft[:, b, 0:CW],
                                     start=first, stop=False,
                                     tile_position=(0, 0))
                    nc.tensor.matmul(acc[C2:P, :], yt, ft[:, b, CW:HW_],
                                     start=first, stop=False,
                                     tile_position=(0, C2))

            def p1_epilogue(h):
                ctx_p = tc.high_priority(offset=600)
                ctx_p.__enter__()
                acc = p1acc[h]
                # x@w2 term, contracted over zero-padded K=128 (stop)
                nc.tensor.matmul(acc[0:C2, :], w2i0p[:],
                                 xm[:, 2 * h * CW:(2 * h + 1) * CW],
                                 start=False, stop=True,
                                 tile_position=(0, 0))
                nc.tensor.matmul(acc[C2:P, :], w2i0p[:],
                                 xm[:, (2 * h + 1) * CW:(2 * h + 2) * CW],
                                 start=False, stop=True,
                                 tile_position=(0, C2))
                xb1 = xbpool.tile([P, CW], BF16, name="xb1")
                nc.scalar.activation(
                    xb1[0:C2, :], acc[0:C2, :],
                    mybir.ActivationFunctionType.Relu,
                    bias=bi0d[0:C2, :], scale=1.0,
                )
                nc.scalar.activation(
                    xb1[C2:P, :], acc[C2:P, :],
                    mybir.ActivationFunctionType.Relu,
                    bias=bi0d[C2:P, :], scale=1.0,
                )
                psy = psmall.tile([P, 4, P], F32, name="psy", tag="ps0")
                for g in range(8):
                    nc.tensor.matmul(
                        psy[(g % 2) * C2:(g % 2) * C2 + C2, g // 2, :],
                        xb1[:, g * C2:(g + 1) * C2], w1i1p[:],
                        start=True, stop=True,
                        tile_position=(0, (g % 2) * C2),
                    )
                y1h = ylpool.tile([P, 8, C2], BF16, name="y1h")
                nc.vector.tensor_copy(y1h[:, 0:4, :], psy[:, :, 0:C2])
                nc.vector.tensor_copy(y1h[:, 4:8, :], psy[:, :, C2:P])
                # p-major gather payload: per-partition-contiguous 1 KiB
                gin = dram.tile([P, 8 * C2], BF16, name="gin", tag="gin",
                                bufs=2)
                nc.scalar.dma_start(
                    gin[:], y1h[:].rearrange("p t ch -> p (t ch)"))
                gout = dram.tile(
                    [NCORES * P, 8 * C2], BF16, name="gout", tag="gout",
                    addr_space="Shared", bufs=2,
                )
                nc.gpsimd.collective_compute(
                    "AllGather", mybir.AluOpType.bypass,
                    replica_groups=RG,
                    ins=[gin[:].opt()], outs=[gout[:].opt()],
                )
                gouts.append(gout)
                ctx_p.__exit__(None, None, None)

            gouts = []
            ygt = [None] * 4
            yg_issued = [False] * 2

            def issue_yg(hg):
                if yg_issued[hg]:
                    return
                with tc.high_priority(offset=600):
                    for hh in range(2):
                        t = xtqpool.tile([P, N // 8], BF16,
                                         name=f"yg{hg}{hh}", tag="xtq")
                        ygt[hg * 2 + hh] = t
                        # SWDGE: rides the gpsimd queue, FIFO behind the
                        # gather it consumes; keeps the ACT ring free for
                        # the next epilogue's gin DMA
                        nc.gpsimd.dma_start(
                            t[:].rearrange("p (m c) -> p m c", c=8 * C2),
                            gouts[hg][hh * 4 * P:(hh * 4 + 4) * P, :]
                            .rearrange("(m p) c -> p m c", p=P),
                        )
                yg_issued[hg] = True

            # ---- pass-2 state
            p2acc = [None, None]   # [chunks 0|1, chunks 2|3]

            def p2_init():
                p2acc[0] = pacc.tile([P, CW], F32, name="p2a", tag="acc")
                p2acc[1] = pacc.tile([P, CW], F32, name="p2b", tag="acc")
                for pair in range(2):
                    for t in range(2):
                        ck = 2 * pair + t
                        nc.tensor.matmul(
                            p2acc[pair][t * C2:(t + 1) * C2, :],
                            w2i1p[:], xm[:, ck * CW:(ck + 1) * CW],
                            start=True, stop=False,
                            tile_position=(0, t * C2),
                        )

            def p2_tile(hg, j, i, ft0, ft1, stop=False):
                b = i % 4
                yt = ygt[hg * 2 + j // 4][
                    :, ((j % 4) * 8 + i) * C2:((j % 4) * 8 + i + 1) * C2]
                nc.tensor.matmul(p2acc[0][0:C2, :], yt, ft0[:, b, 0:CW],
                                 start=False, stop=stop,
                                 tile_position=(0, 0))
                nc.tensor.matmul(p2acc[0][C2:P, :], yt, ft0[:, b, CW:HW_],
                                 start=False, stop=stop,
                                 tile_position=(0, C2))
                if ft1 is None:
                    return
                nc.tensor.matmul(p2acc[1][0:C2, :], yt, ft1[:, b, 0:CW],
                                 start=False, stop=stop,
                                 tile_position=(0, 0))
                nc.tensor.matmul(p2acc[1][C2:P, :], yt, ft1[:, b, CW:HW_],
                                 start=False, stop=stop,
                                 tile_position=(0, C2))

            def p2_stream_block(hg, j, blk_i):
                # DMA half-array 0/1 blocks for (j, i in [4*blk_i,+4))
                ktb = j * 4 + 2 * hg + blk_i
                fts = []
                for ha in range(2):
                    ft = kept.get((ha, ktb))
                    if ft is None:
                        ft = fpool.tile([P, KB1, HW_], F8, name="ft2",
                                        tag="ft")
                        nc.sync.dma_start(
                            ft[:],
                            fltr_halves[ha][ktb * P:(ktb + 1) * P, :]
                            .rearrange("p (b c) -> p b c", c=HW_),
                        )
                    fts.append(ft)
                return fts

            def p2_epilogue(pair):
                ctx_p = tc.high_priority(offset=600)
                ctx_p.__enter__()
                acc = p2acc[pair]
                xb2 = xbpool.tile([P, CW], BF16, name="xb2")
                for t in range(2):
                    nc.scalar.activation(
                        xb2[t * C2:(t + 1) * C2, :],
                        acc[t * C2:(t + 1) * C2, :],
                        mybir.ActivationFunctionType.Relu,
                        bias=bi1hd[t * C2:(t + 1) * C2, :], scale=0.5,
                    )
                pso = psmall.tile([P, CW], F32, name="pso", tag="ps0")
                for t in range(2):
                    nc.tensor.matmul(
                        pso[t * C2:(t + 1) * C2, :],
                        ssum[:, t * C2:(t + 1) * C2],
                        xb2[:],
                        start=True, stop=True,
                        tile_position=(0, t * C2),
                    )
                oT = xbpool.tile([P, CW], F32, name="oT")
                nc.vector.tensor_copy(oT[:], pso[:])
                for t in range(2):
                    ck = 2 * pair + t
                    nc.scalar.dma_start(
                        out_e[:, ck * CW:(ck + 1) * CW],
                        oT[t * C2:t * C2 + C, :],
                    )
                ctx_p.__exit__(None, None, None)

            # ================= emission =================
            # pass-1 half 0 (+ Y0 interleaved)
            _mark(nc, "p1h0_start")
            p1acc[0] = pacc.tile([P, CW], F32, name="p1a", tag="acc")
            for ktb in range(NBLK):
                if ktb < 16:
                    y0_block(ktb)
                ft = p1_dma_block(0, ktb)
                p1_mms(0, ktb, ft)
            _mark(nc, "p1h0_end")
            p1_epilogue(0)
            _mark(nc, "epi0_end")

            # pass-1 half 1; yg0 load + p2 acc init issued mid-half so
            # phase A can start the moment half-1's matmuls finish
            import os as _os
            ilv = _os.environ.get("ARMA_ILV") == "1"
            jilv = 3 if ilv else 0
            p1acc[1] = pacc.tile([P, CW], F32, name="p1b", tag="acc")
            for ktb in range(NBLK):
                ft = p1_dma_block(1, ktb)
                p1_mms(1, ktb, ft)
                if ktb == 8:
                    issue_yg(0)
                if ktb == 12:
                    p2_init()
                if ilv and ktb >= 16 and ktb % 4 == 0 and (ktb - 16) // 4 < 3:
                    j = (ktb - 16) // 4
                    for i in range(8):
                        ktb2 = j * 4 + i // 4
                        p2_tile(0, j, i, kept[(0, ktb2)], kept[(1, ktb2)])
            _mark(nc, "p1h1_end")
            p1_epilogue(1)
            _mark(nc, "epi1_end")
            issue_yg(1)

            # pinned phase A (j = jilv..JPIN-1)
            for j in range(jilv, JPIN):
                for i in range(8):
                    ktb2 = j * 4 + i // 4
                    p2_tile(0, j, i, kept[(0, ktb2)], kept[(1, ktb2)])
            _mark(nc, "phA_pinned_end")
            # streamed phase A (j = JPIN..7)
            for j in range(JPIN, NCORES):
                for blk_i in range(2):
                    fts = p2_stream_block(0, j, blk_i)
                    for i in range(4 * blk_i, 4 * blk_i + 4):
                        p2_tile(0, j, i, fts[0], fts[1])
            _mark(nc, "phA_end")
            # phase B (gather half 1); last block split for epilogue hiding
            for j in range(NCORES):
                for blk_i in range(2):
                    lastblk = j == NCORES - 1 and blk_i == 1
                    fts = p2_stream_block(1, j, blk_i)
                    if not lastblk:
                        for i in range(4 * blk_i, 4 * blk_i + 4):
                            p2_tile(1, j, i, fts[0], fts[1])
                    else:
                        for i in range(4, 8):
                            # chunks 0,1 first; stop their groups
                            b = i % 4
                            yt = ygt[2 + j // 4][
                                :, ((j % 4) * 8 + i) * C2:
                                ((j % 4) * 8 + i + 1) * C2]
                            nc.tensor.matmul(
                                p2acc[0][0:C2, :], yt, fts[0][:, b, 0:CW],
                                start=False, stop=(i == 7),
                                tile_position=(0, 0))
                            nc.tensor.matmul(
                                p2acc[0][C2:P, :], yt, fts[0][:, b, CW:HW_],
                                start=False, stop=(i == 7),
                                tile_position=(0, C2))
                        p2_epilogue(0)
                        for i in range(4, 8):
                            b = i % 4
                            yt = ygt[2 + j // 4][
                                :, ((j % 4) * 8 + i) * C2:
                                ((j % 4) * 8 + i + 1) * C2]
                            nc.tensor.matmul(
                                p2acc[1][0:C2, :], yt, fts[1][:, b, 0:CW],
                                start=False, stop=(i == 7),
                                tile_position=(0, 0))
                            nc.tensor.matmul(
                                p2acc[1][C2:P, :], yt, fts[1][:, b, CW:HW_],
                                start=False, stop=(i == 7),
                                tile_position=(0, C2))
                        p2_epilogue(1)

    nc.compile()
    return nc


def kernel(**inputs):
    x = np.ascontiguousarray(np.asarray(inputs["x"], dtype=np.float32))
    fltr = np.ascontiguousarray(np.asarray(inputs["fltr"], dtype=np.float32))

    def cat(a, b, axis=1):
        return np.ascontiguousarray(
            np.concatenate(
                [np.asarray(a, np.float32), np.asarray(b, np.float32)],
                axis=axis,
            )
        )

    f8 = ml_dtypes.float8_e3m4
    bf = ml_dtypes.bfloat16

    # fused conv kernels, descaled by 2^-8 (fp8 fold)
    w1i0f = (cat(inputs["k0i0_w1"], inputs["k1i0_w1"]) / FSCALE)  # [64,64]
    w1i1f = np.zeros((C2, C2), dtype=np.float32)
    w1i1f[0:C, 0:C] = np.asarray(inputs["k0i1_w1"], np.float32)
    w1i1f[C:C2, C:C2] = np.asarray(inputs["k1i1_w1"], np.float32)
    w1i1f = w1i1f / FSCALE

    def dpad(w):  # [[w,0],[0,w]] -> [128, 128]
        o = np.zeros((P, P), dtype=np.float32)
        o[0:C2, 0:C2] = w
        o[C2:P, C2:P] = w
        return o

    w1i0p = np.ascontiguousarray(dpad(w1i0f).astype(bf))
    w1i1p = np.ascontiguousarray(dpad(w1i1f).astype(bf))

    def zpad(w):  # [w; 0] -> [128, 64]
        o = np.zeros((P, C2), dtype=np.float32)
        o[0:C2, :] = w
        return np.ascontiguousarray(o)

    w2i0p = zpad(cat(inputs["k0i0_w2"], inputs["k1i0_w2"])).astype(bf)
    w2i1p = zpad(cat(inputs["k0i1_w2"], inputs["k1i1_w2"])).astype(bf)
    bi0d = np.ascontiguousarray(
        np.tile(cat(inputs["k0i0_b"], inputs["k1i0_b"], axis=0), 2)[:, None])
    bi1hd = np.ascontiguousarray(
        0.5 * np.tile(cat(inputs["k0i1_b"], inputs["k1i1_b"], axis=0),
                      2)[:, None])

    # stack-mean selection: out[m] = xb2[m] + xb2[m+32] for m < 32
    ssum = np.zeros((P, P), dtype=np.float32)
    for m in range(C):
        ssum[m, m] = ssum[m + C, m] = 1.0
        ssum[C2 + m, C2 + m] = ssum[C2 + C + m, C2 + m] = 1.0
    ssum = np.ascontiguousarray(ssum.astype(bf))

    xT = x.T.astype(bf)                      # [64, 16384]
    xt = np.ascontiguousarray(
        np.concatenate([xT[:, 0:N // 2], xT[:, N // 2:N]], axis=0))

    # fp8 E3M4 fltr at rest, transposed per core, scaled by 2^8
    fltrs = (fltr * np.float32(FSCALE)).astype(f8)

    if "nc" not in _CACHE:
        _CACHE["nc"] = _build()
    nc = _CACHE["nc"]

    in_maps = []
    for m in range(NCORES):
        rows = slice(m * R, (m + 1) * R)
        xtm = np.zeros((P, R), dtype=np.float32)
        xtm[0:F, :] = x[rows, :].T
        xtm = xtm.astype(bf)
        def blk(a):  # [N, HW_] -> [NBLK*P, KB1*HW_] block-interleaved
            return np.ascontiguousarray(
                a.reshape(NBLK, KB1, P, HW_).transpose(0, 2, 1, 3)
                .reshape(NBLK * P, KB1 * HW_))

        in_maps.append({
            "fltrt0": blk(np.ascontiguousarray(
                fltrs[m * R:m * R + HW_, :].T)),
            "fltrt1": blk(np.ascontiguousarray(
                fltrs[m * R + HW_:(m + 1) * R, :].T)),
            "xt": xt,
            "xtm": np.ascontiguousarray(xtm),
            "w1i0p": w1i0p, "w1i1p": w1i1p,
            "w2i0p": w2i0p, "w2i1p": w2i1p,
            "bi0d": bi0d, "bi1hd": bi1hd, "ssum": ssum,
        })

    import os
    import time
    trace = os.environ.get("ARMA_TRACE") == "1"
    last_exc = None
    for attempt in range(3):
        try:
            res = run_bass_kernel_spmd(
                nc, in_maps, core_ids=list(range(NCORES)), trace=trace,
            )
            break
        except Exception as e:  # transient NRT device errors: retry
            last_exc = e
            time.sleep(5.0)
    else:
        raise last_exc
    _CACHE["last_results"] = res
    out = np.concatenate(
        [np.asarray(res.results[m]["out"]).T for m in range(NCORES)], axis=0
    )
    return out


# revision 18
# speedup vs baseline: 1.2835x; 1.0233x over previous
"""Distributed ARMAConv kernel for 8 TRN2 NeuronCores (Bass/Tile).

Reference computation (N=16384 nodes, F=64 in-feats, C=32 channels,
K=2 stacks, T=2 iterations):
    for each stack k:  xbar = x
        for i in 0..1: xbar = relu(fltr @ (xbar @ w1) + x @ w2 + b)
    out = mean over stacks                                  -> [N, 32]

Strategy (v2 - 2x column-tiled PE):
  - Row-shard fltr across 8 cores; core m holds fltr[rows_m, :] stored
    TRANSPOSED as two contiguous half-arrays (1 KiB DMA lines), fp8
    E3M4 at rest, pre-scaled by 2^8 (descale folded into w1).
  - Fuse the two ARMA stacks: Y = [xbar@w1_k0 | xbar@w1_k1] is [N,64],
    so fltr streams once per iteration.
  - The stationary operand (Y tile [128,64]) only fills half the
    128-wide PE array.  ALL matmuls run 2x column-tiled (tile_size
    (128,64)): tile (0,0) -> PSUM partitions 0-63, tile (0,64) ->
    64-127, each with its own moving fltr stream.  Measured 519 ns per
    kt-tile (2 concurrent 512-wide fp8 streams) vs 1034 serial -> PE
    ~67us per pass instead of ~110, making the kernel DMA-bound.
  - Every matmul in the kernel keeps tile_size (128,64) (no mode
    switches): K=64 matmuls (Y0 = x@w1, Y1 = relu@w1i1, w2-terms,
    final stack-mean) are zero-padded to K=128, with zeros placed in
    the operand that multiplies the junk rows.  The final stack-mean
    (out = 0.5*(relu_lo + relu_hi)) runs on the PE via a 0/1
    selection stationary instead of partition-shift DMA + DVE add.
  - The x@w2 bias term is accumulated LAST (stop) instead of first,
    so xm's DMA is off the critical path; the first fltr matmul
    carries start=True.
  - SBUF pinning: the pass-1 fltr blocks that pass-2 phase A (gather
    half 0, low core-blocks) consumes are kept resident (JPIN blocks
    per half-array stream); phase-A matmuls for those blocks are
    interleaved into pass-1 half-1's DMA-bound stream, filling PE
    idle slots.  Pass 2 re-reads only the rest.
  - Big fltr DMAs ride the sync-engine HWDGE ring; small/latency DMAs
    ride the scalar-engine ring; collectives keep the gpsimd queue.
    A dummy warm-up collective at t=0 absorbs the one-time rendezvous
    barrier (~60us) that would otherwise delay gather 0.
"""

import numpy as np
import ml_dtypes

import concourse.mybir as mybir
import concourse.tile as tile
from concourse import bacc
from concourse.bass_utils import run_bass_kernel_spmd

N = 16384            # nodes
F = 64               # input features
C = 32               # channels per stack
C2 = 2 * C           # fused channels (2 stacks)
NCORES = 8
R = N // NCORES      # fltr rows per core (2048)
P = 128              # partitions
NKT = N // P         # kt tiles per full pass (128)
HW_ = R // 2         # 1024 output rows per half-array
CW = 512             # output rows per chunk / PSUM accumulator slice
KB1 = 4              # kt tiles per fltr DMA block (512 KiB)
NBLK = NKT // KB1    # 32 blocks per half-array
JPIN = 6             # pin blocks {4j,4j+1: j<JPIN} of both half-arrays
FSCALE = 256.0       # power-of-2 fp8 pre-scale (folded into w1)

F32 = mybir.dt.float32
F32R = mybir.dt.float32r
BF16 = mybir.dt.bfloat16
F8 = mybir.dt.float8e3

_CACHE = {}


PHASES = []


def _mark(nc, label):
    PHASES.append((label, sum(1 for _ in nc.all_instructions())))


def _build():
    nc = bacc.Bacc(
        trn_type="TRN2", target_bir_lowering=False, debug=False,
        num_devices=NCORES,
    )
    # block-interleaved fltr: row ktb*128+p holds that partition's 4
    # kt-rows contiguously -> 4 KiB DMA descriptors (near-ceiling HBM rate)
    fltrT0_e = nc.dram_tensor("fltrt0", [NBLK * P, KB1 * HW_], F8,
                              kind="ExternalInput")
    fltrT1_e = nc.dram_tensor("fltrt1", [NBLK * P, KB1 * HW_], F8,
                              kind="ExternalInput")
    xt_e = nc.dram_tensor("xt", [P, N // 2], BF16, kind="ExternalInput")
    xtm_e = nc.dram_tensor("xtm", [P, R], BF16, kind="ExternalInput")
    w1i0_e = nc.dram_tensor("w1i0p", [P, P], BF16, kind="ExternalInput")
    w1i1_e = nc.dram_tensor("w1i1p", [P, P], BF16, kind="ExternalInput")
    w2i0_e = nc.dram_tensor("w2i0p", [P, C2], BF16, kind="ExternalInput")
    w2i1_e = nc.dram_tensor("w2i1p", [P, C2], BF16, kind="ExternalInput")
    bi0_e = nc.dram_tensor("bi0d", [P, 1], F32, kind="ExternalInput")
    bi1h_e = nc.dram_tensor("bi1hd", [P, 1], F32, kind="ExternalInput")
    ssum_e = nc.dram_tensor("ssum", [P, P], BF16, kind="ExternalInput")
    out_e = nc.dram_tensor("out", [C, R], F32, kind="ExternalOutput")

    RG = [list(range(NCORES))]
    fltr_halves = [fltrT0_e, fltrT1_e]

    with tile.TileContext(nc) as tc:
        with (
            tc.tile_pool(name="wpool", bufs=1) as wpool,
            tc.tile_pool(name="y0pool", bufs=1) as y0pool,
            tc.tile_pool(name="xtq", bufs=4) as xtqpool,
            tc.tile_pool(name="kpool", bufs=1) as kpool,
            tc.tile_pool(name="fpool", bufs=10) as fpool,
            tc.tile_pool(name="xbpool", bufs=2) as xbpool,
            tc.tile_pool(name="ylpool", bufs=2) as ylpool,
            tc.tile_pool(name="pacc", bufs=4, space="PSUM") as pacc,
            tc.tile_pool(name="psmall", bufs=2, space="PSUM") as psmall,
            tc.tile_pool(name="dram", bufs=8, space="DRAM") as dram,
        ):
            # ---- resident small tensors (scalar ring) + xt (sync ring,
            # ---- ahead of the fltr stream; 4 independent tiles)
            w1i0p = wpool.tile([P, P], BF16)
            nc.scalar.dma_start(w1i0p[:], w1i0_e[:])
            xts = []
            for q in range(4):
                xq = xtqpool.tile([P, N // 8], BF16, name=f"xt{q}",
                                  tag="xtq")
                nc.scalar.dma_start(xq[:],
                                    xt_e[:, q * (N // 8):(q + 1) * (N // 8)])
                xts.append(xq)

            # dummy warm-up collective: anchors the one-time rendezvous
            # barrier while all cores are still in startup.
            gwin = dram.tile([F, C2], BF16, name="gwin", tag="gwin")
            nc.scalar.dma_start(gwin[:], w1i0p[0:F, 0:C2])
            gwout = dram.tile([NCORES * F, C2], BF16, name="gwout",
                              tag="gwout", addr_space="Shared")
            nc.gpsimd.collective_compute(
                "AllGather", mybir.AluOpType.bypass,
                replica_groups=RG,
                ins=[gwin[:].opt()], outs=[gwout[:].opt()],
            )

            w1i1p = wpool.tile([P, P], BF16)
            nc.scalar.dma_start(w1i1p[:], w1i1_e[:])
            w2i0p = wpool.tile([P, C2], BF16)
            nc.scalar.dma_start(w2i0p[:], w2i0_e[:])
            w2i1p = wpool.tile([P, C2], BF16)
            nc.scalar.dma_start(w2i1p[:], w2i1_e[:])
            bi0d = wpool.tile([P, 1], F32)
            nc.scalar.dma_start(bi0d[:], bi0_e[:])
            bi1hd = wpool.tile([P, 1], F32)
            nc.scalar.dma_start(bi1hd[:], bi1h_e[:])
            ssum = wpool.tile([P, P], BF16)
            nc.scalar.dma_start(ssum[:], ssum_e[:])
            xm = wpool.tile([P, R], BF16)
            nc.scalar.dma_start(xm[:], xtm_e[:])

            y0 = y0pool.tile([P, NKT * C2], BF16, tag="y0")

            def y0_block(b):
                # twin-node: one N=128 matmul computes Y0 for a 64-node
                # lo-half group (xt rows 0-63 x w1i0p cols 0:64) AND its
                # hi-half twin (rows 64-127 x cols 64:128).  Block b
                # covers lo nodes [512b, 512b+512) = lo kt 4b..4b+4 and
                # their twins kt 64+4b..; psum placement makes the
                # evacuation one contiguous [128, 512] copy.
                ps = psmall.tile([P, 4, P], F32, name="ps0", tag="ps0")
                q4 = b // 4
                base = b * 512 - q4 * (N // 8)
                xq = xts[q4]
                for q in range(8):
                    col = base + q * C2
                    nc.tensor.matmul(
                        ps[(q % 2) * C2:(q % 2) * C2 + C2, q // 2, :],
                        xq[:, col:col + C2], w1i0p[:],
                        start=True, stop=True,
                        tile_position=(0, (q % 2) * C2),
                    )
                nc.vector.tensor_copy(
                    y0[:, b * CW:(b + 1) * CW],
                    ps[:].rearrange("p u c -> p (u c)"))

            # ---- pass-1 state
            p1acc = [None, None]
            kept = {}

            def p1_dma_block(h, ktb):
                if ktb % 4 < 2 and ktb // 4 < JPIN:
                    ft = kpool.tile([P, KB1, HW_], F8, name="ftk",
                                    tag="ftk", bufs=4 * JPIN)
                    kept[(h, ktb)] = ft
                else:
                    ft = fpool.tile([P, KB1, HW_], F8, name="ft", tag="ft")
                nc.sync.dma_start(
                    ft[:],
                    fltr_halves[h][ktb * P:(ktb + 1) * P, :]
                    .rearrange("p (b c) -> p b c", c=HW_),
                )
                return ft

            def y0sl(kt):
                if kt < NKT // 2:
                    return y0[:, kt * P:kt * P + C2]
                return y0[:, (kt - NKT // 2) * P + C2:(kt - NKT // 2) * P + P]

            def p1_mms(h, ktb, ft):
                acc = p1acc[h]
                for b in range(KB1):
                    kt = ktb * KB1 + b
                    first = kt == 0
                    yt = y0sl(kt)
                    nc.tensor.matmul(acc[0:C2, :], yt, ft[:, b, 0:CW],
                                     start=first, stop=False,
                                     tile_position=(0, 0))
                    nc.tensor.matmul(acc[C2:P, :], yt, ft[:, b, CW:HW_],
                                     start=first, stop=False,
                                     tile_position=(0, C2))

            def p1_epilogue(h):
                ctx_p = tc.high_priority(offset=600)
                ctx_p.__enter__()
                acc = p1acc[h]
                # x@w2 term, contracted over zero-padded K=128 (stop)
                nc.tensor.matmul(acc[0:C2, :], w2i0p[:],
                                 xm[:, 2 * h * CW:(2 * h + 1) * CW],
                                 start=False, stop=True,
                                 tile_position=(0, 0))
                nc.tensor.matmul(acc[C2:P, :], w2i0p[:],
                                 xm[:, (2 * h + 1) * CW:(2 * h + 2) * CW],
                                 start=False, stop=True,
                                 tile_position=(0, C2))
                xb1 = xbpool.tile([P, CW], BF16, name="xb1")
                nc.scalar.activation(
                    xb1[0:C2, :], acc[0:C2, :],
                    mybir.ActivationFunctionType.Relu,
                    bias=bi0d[0:C2, :], scale=1.0,
                )
                nc.scalar.activation(
                    xb1[C2:P, :], acc[C2:P, :],
                    mybir.ActivationFunctionType.Relu,
                    bias=bi0d[C2:P, :], scale=1.0,
                )
                psy = psmall.tile([P, 4, P], F32, name="psy", tag="ps0")
                for g in range(8):
                    nc.tensor.matmul(
                        psy[(g % 2) * C2:(g % 2) * C2 + C2, g // 2, :],
                        xb1[:, g * C2:(g + 1) * C2], w1i1p[:],
                        start=True, stop=True,
                        tile_position=(0, (g % 2) * C2),
                    )
                y1h = ylpool.tile([P, 8, C2], BF16, name="y1h")
                nc.vector.tensor_copy(y1h[:, 0:4, :], psy[:, :, 0:C2])
                nc.vector.tensor_copy(y1h[:, 4:8, :], psy[:, :, C2:P])
                # two sub-gathers per half (t 0..3 / 4..7) so pass-2 can
                # start on the first 64 KiB while the second is in flight
                for part in range(2):
                    gin = dram.tile([P, 4 * C2], BF16, name=f"gin{part}",
                                    tag=f"gin{part}", bufs=2)
                    nc.scalar.dma_start(
                        gin[:],
                        y1h[:, 4 * part:4 * part + 4, :]
                        .rearrange("p t ch -> p (t ch)"))
                    gout = dram.tile(
                        [NCORES * P, 4 * C2], BF16, name=f"gout{part}",
                        tag=f"gout{part}", addr_space="Shared", bufs=2,
                    )
                    nc.gpsimd.collective_compute(
                        "AllGather", mybir.AluOpType.bypass,
                        replica_groups=RG,
                        ins=[gin[:].opt()], outs=[gout[:].opt()],
                    )
                    gouts.append(gout)
                ctx_p.__exit__(None, None, None)

            gouts = []
            ygt = [None] * 4
            yg_issued = [False] * 2

            def issue_yg(hg):
                if yg_issued[hg]:
                    return
                with tc.high_priority(offset=600):
                    for part in range(2):
                        t = xtqpool.tile([P, N // 8], BF16,
                                         name=f"yg{hg}{part}", tag="xtq")
                        ygt[hg * 2 + part] = t
                        # SWDGE: rides the gpsimd queue, FIFO behind the
                        # gather it consumes; keeps the ACT ring free for
                        # the next epilogue's gin DMA
                        nc.gpsimd.dma_start(
                            t[:].rearrange("p (m c) -> p m c", c=4 * C2),
                            gouts[hg * 2 + part][:]
                            .rearrange("(m p) c -> p m c", p=P),
                        )
                yg_issued[hg] = True

            # ---- pass-2 state
            p2acc = [None, None]   # [chunks 0|1, chunks 2|3]

            def p2_init():
                p2acc[0] = pacc.tile([P, CW], F32, name="p2a", tag="acc")
                p2acc[1] = pacc.tile([P, CW], F32, name="p2b", tag="acc")
                for pair in range(2):
                    for t in range(2):
                        ck = 2 * pair + t
                        nc.tensor.matmul(
                            p2acc[pair][t * C2:(t + 1) * C2, :],
                            w2i1p[:], xm[:, ck * CW:(ck + 1) * CW],
                            start=True, stop=False,
                            tile_position=(0, t * C2),
                        )

            def p2_tile(hg, j, i, ft0, ft1, stop=False):
                b = i % 4
                yt = ygt[hg * 2 + i // 4][
                    :, (j * 4 + i % 4) * C2:(j * 4 + i % 4 + 1) * C2]
                nc.tensor.matmul(p2acc[0][0:C2, :], yt, ft0[:, b, 0:CW],
                                 start=False, stop=stop,
                                 tile_position=(0, 0))
                nc.tensor.matmul(p2acc[0][C2:P, :], yt, ft0[:, b, CW:HW_],
                                 start=False, stop=stop,
                                 tile_position=(0, C2))
                if ft1 is None:
                    return
                nc.tensor.matmul(p2acc[1][0:C2, :], yt, ft1[:, b, 0:CW],
                                 start=False, stop=stop,
                                 tile_position=(0, 0))
                nc.tensor.matmul(p2acc[1][C2:P, :], yt, ft1[:, b, CW:HW_],
                                 start=False, stop=stop,
                                 tile_position=(0, C2))

            def p2_stream_block(hg, j, blk_i):
                # DMA half-array 0/1 blocks for (j, i in [4*blk_i,+4))
                ktb = j * 4 + 2 * hg + blk_i
                fts = []
                for ha in range(2):
                    ft = kept.get((ha, ktb))
                    if ft is None:
                        ft = fpool.tile([P, KB1, HW_], F8, name="ft2",
                                        tag="ft")
                        nc.sync.dma_start(
                            ft[:],
                            fltr_halves[ha][ktb * P:(ktb + 1) * P, :]
                            .rearrange("p (b c) -> p b c", c=HW_),
                        )
                    fts.append(ft)
                return fts

            def p2_epilogue(pair):
                ctx_p = tc.high_priority(offset=600)
                ctx_p.__enter__()
                acc = p2acc[pair]
                xb2 = xbpool.tile([P, CW], BF16, name="xb2")
                for t in range(2):
                    nc.scalar.activation(
                        xb2[t * C2:(t + 1) * C2, :],
                        acc[t * C2:(t + 1) * C2, :],
                        mybir.ActivationFunctionType.Relu,
                        bias=bi1hd[t * C2:(t + 1) * C2, :], scale=0.5,
                    )
                pso = psmall.tile([P, CW], F32, name="pso", tag="ps0")
                for t in range(2):
                    nc.tensor.matmul(
                        pso[t * C2:(t + 1) * C2, :],
                        ssum[:, t * C2:(t + 1) * C2],
                        xb2[:],
                        start=True, stop=True,
                        tile_position=(0, t * C2),
                    )
                oT = xbpool.tile([P, CW], F32, name="oT")
                nc.vector.tensor_copy(oT[:], pso[:])
                for t in range(2):
                    ck = 2 * pair + t
                    nc.scalar.dma_start(
                        out_e[:, ck * CW:(ck + 1) * CW],
                        oT[t * C2:t * C2 + C, :],
                    )
                ctx_p.__exit__(None, None, None)

            # ================= emission =================
            # pass-1 half 0 (+ Y0 interleaved)
            _mark(nc, "p1h0_start")
            p1acc[0] = pacc.tile([P, CW], F32, name="p1a", tag="acc")
            for ktb in range(NBLK):
                if ktb < 16:
                    y0_block(ktb)
                ft = p1_dma_block(0, ktb)
                p1_mms(0, ktb, ft)
            _mark(nc, "p1h0_end")
            p1_epilogue(0)
            _mark(nc, "epi0_end")

            # pass-1 half 1; yg0 load + p2 acc init issued mid-half so
            # phase A can start the moment half-1's matmuls finish
            import os as _os
            ilv = _os.environ.get("ARMA_ILV") == "1"
            jilv = 3 if ilv else 0
            p1acc[1] = pacc.tile([P, CW], F32, name="p1b", tag="acc")
            for ktb in range(NBLK):
                ft = p1_dma_block(1, ktb)
                p1_mms(1, ktb, ft)
                if ktb == 8:
                    issue_yg(0)
                if ktb == 12:
                    p2_init()
                if ilv and ktb >= 16 and ktb % 4 == 0 and (ktb - 16) // 4 < 3:
                    j = (ktb - 16) // 4
                    for i in range(8):
                        ktb2 = j * 4 + i // 4
                        p2_tile(0, j, i, kept[(0, ktb2)], kept[(1, ktb2)])
            _mark(nc, "p1h1_end")
            p1_epilogue(1)
            _mark(nc, "epi1_end")
            issue_yg(1)

            # phase A part-major: all i<4 tiles (sub-gather a), then i>=4
            for blk_i in range(2):
                for j in range(jilv, JPIN):
                    ktb2 = j * 4 + blk_i
                    for i in range(4 * blk_i, 4 * blk_i + 4):
                        p2_tile(0, j, i, kept[(0, ktb2)], kept[(1, ktb2)])
                if blk_i == 1:
                    continue
                for j in range(JPIN, NCORES):
                    fts = p2_stream_block(0, j, 0)
                    for i in range(0, 4):
                        p2_tile(0, j, i, fts[0], fts[1])
            _mark(nc, "phA_pinned_end")
            for j in range(JPIN, NCORES):
                fts = p2_stream_block(0, j, 1)
                for i in range(4, 8):
                    p2_tile(0, j, i, fts[0], fts[1])
            _mark(nc, "phA_end")
            # phase B part-major; last block split for epilogue hiding
            for blk_i in range(2):
                for j in range(NCORES):
                    lastblk = j == NCORES - 1 and blk_i == 1
                    if lastblk:
                        continue
                    fts = p2_stream_block(1, j, blk_i)
                    for i in range(4 * blk_i, 4 * blk_i + 4):
                        p2_tile(1, j, i, fts[0], fts[1])
            # last block (j=7, part 1): chunks 0,1 finish + epilogue
            # while chunks 2,3 still accumulate, hiding the epilogue
            j = NCORES - 1
            fts = p2_stream_block(1, j, 1)
            for i in range(4, 8):
                b = i % 4
                yt = ygt[3][:, (j * 4 + i % 4) * C2:
                            (j * 4 + i % 4 + 1) * C2]
                nc.tensor.matmul(
                    p2acc[0][0:C2, :], yt, fts[0][:, b, 0:CW],
                    start=False, stop=(i == 7), tile_position=(0, 0))
                nc.tensor.matmul(
                    p2acc[0][C2:P, :], yt, fts[0][:, b, CW:HW_],
                    start=False, stop=(i == 7), tile_position=(0, C2))
            p2_epilogue(0)
            for i in range(4, 8):
                b = i % 4
                yt = ygt[3][:, (j * 4 + i % 4) * C2:
                            (j * 4 + i % 4 + 1) * C2]
                nc.tensor.matmul(
                    p2acc[1][0:C2, :], yt, fts[1][:, b, 0:CW],
                    start=False, stop=(i == 7), tile_position=(0, 0))
                nc.tensor.matmul(
                    p2acc[1][C2:P, :], yt, fts[1][:, b, CW:HW_],
                    start=False, stop=(i == 7), tile_position=(0, C2))
            p2_epilogue(1)

    nc.compile()
    return nc


def kernel(**inputs):
    x = np.ascontiguousarray(np.asarray(inputs["x"], dtype=np.float32))
    fltr = np.ascontiguousarray(np.asarray(inputs["fltr"], dtype=np.float32))

    def cat(a, b, axis=1):
        return np.ascontiguousarray(
            np.concatenate(
                [np.asarray(a, np.float32), np.asarray(b, np.float32)],
                axis=axis,
            )
        )

    f8 = ml_dtypes.float8_e3m4
    bf = ml_dtypes.bfloat16

    # fused conv kernels, descaled by 2^-8 (fp8 fold)
    w1i0f = (cat(inputs["k0i0_w1"], inputs["k1i0_w1"]) / FSCALE)  # [64,64]
    w1i1f = np.zeros((C2, C2), dtype=np.float32)
    w1i1f[0:C, 0:C] = np.asarray(inputs["k0i1_w1"], np.float32)
    w1i1f[C:C2, C:C2] = np.asarray(inputs["k1i1_w1"], np.float32)
    w1i1f = w1i1f / FSCALE

    def dpad(w):  # [[w,0],[0,w]] -> [128, 128]
        o = np.zeros((P, P), dtype=np.float32)
        o[0:C2, 0:C2] = w
        o[C2:P, C2:P] = w
        return o

    w1i0p = np.ascontiguousarray(dpad(w1i0f).astype(bf))
    w1i1p = np.ascontiguousarray(dpad(w1i1f).astype(bf))

    def zpad(w):  # [w; 0] -> [128, 64]
        o = np.zeros((P, C2), dtype=np.float32)
        o[0:C2, :] = w
        return np.ascontiguousarray(o)

    w2i0p = zpad(cat(inputs["k0i0_w2"], inputs["k1i0_w2"])).astype(bf)
    w2i1p = zpad(cat(inputs["k0i1_w2"], inputs["k1i1_w2"])).astype(bf)
    bi0d = np.ascontiguousarray(
        np.tile(cat(inputs["k0i0_b"], inputs["k1i0_b"], axis=0), 2)[:, None])
    bi1hd = np.ascontiguousarray(
        0.5 * np.tile(cat(inputs["k0i1_b"], inputs["k1i1_b"], axis=0),
                      2)[:, None])

    # stack-mean selection: out[m] = xb2[m] + xb2[m+32] for m < 32
    ssum = np.zeros((P, P), dtype=np.float32)
    for m in range(C):
        ssum[m, m] = ssum[m + C, m] = 1.0
        ssum[C2 + m, C2 + m] = ssum[C2 + C + m, C2 + m] = 1.0
    ssum = np.ascontiguousarray(ssum.astype(bf))

    xT = x.T.astype(bf)                      # [64, 16384]
    xt = np.ascontiguousarray(
        np.concatenate([xT[:, 0:N // 2], xT[:, N // 2:N]], axis=0))

    # fp8 E3M4 fltr at rest, transposed per core, scaled by 2^8
    fltrs = (fltr * np.float32(FSCALE)).astype(f8)

    if "nc" not in _CACHE:
        _CACHE["nc"] = _build()
    nc = _CACHE["nc"]

    in_maps = []
    for m in range(NCORES):
        rows = slice(m * R, (m + 1) * R)
        xtm = np.zeros((P, R), dtype=np.float32)
        xtm[0:F, :] = x[rows, :].T
        xtm = xtm.astype(bf)
        def blk(a):  # [N, HW_] -> [NBLK*P, KB1*HW_] block-interleaved
            return np.ascontiguousarray(
                a.reshape(NBLK, KB1, P, HW_).transpose(0, 2, 1, 3)
                .reshape(NBLK * P, KB1 * HW_))

        in_maps.append({
            "fltrt0": blk(np.ascontiguousarray(
                fltrs[m * R:m * R + HW_, :].T)),
            "fltrt1": blk(np.ascontiguousarray(
                fltrs[m * R + HW_:(m + 1) * R, :].T)),
            "xt": xt,
            "xtm": np.ascontiguousarray(xtm),
            "w1i0p": w1i0p, "w1i1p": w1i1p,
            "w2i0p": w2i0p, "w2i1p": w2i1p,
            "bi0d": bi0d, "bi1hd": bi1hd, "ssum": ssum,
        })

    import os
    import time
    trace = os.environ.get("ARMA_TRACE") == "1"
    last_exc = None
    for attempt in range(3):
        try:
            res = run_bass_kernel_spmd(
                nc, in_maps, core_ids=list(range(NCORES)), trace=trace,
            )
            break
        except Exception as e:  # transient NRT device errors: retry
            last_exc = e
            time.sleep(5.0)
    else:
        raise last_exc
    _CACHE["last_results"] = res
    out = np.concatenate(
        [np.asarray(res.results[m]["out"]).T for m in range(NCORES)], axis=0
    )
    return out


# revision 20
# speedup vs baseline: 1.4658x; 1.1421x over previous
"""Distributed ARMAConv kernel for 8 TRN2 NeuronCores (Bass/Tile).

Reference computation (N=16384 nodes, F=64 in-feats, C=32 channels,
K=2 stacks, T=2 iterations):
    for each stack k:  xbar = x
        for i in 0..1: xbar = relu(fltr @ (xbar @ w1) + x @ w2 + b)
    out = mean over stacks                                  -> [N, 32]

Strategy (v2 - 2x column-tiled PE):
  - Row-shard fltr across 8 cores; core m holds fltr[rows_m, :] stored
    TRANSPOSED as two contiguous half-arrays (1 KiB DMA lines), fp8
    E3M4 at rest, pre-scaled by 2^8 (descale folded into w1).
  - Fuse the two ARMA stacks: Y = [xbar@w1_k0 | xbar@w1_k1] is [N,64],
    so fltr streams once per iteration.
  - The stationary operand (Y tile [128,64]) only fills half the
    128-wide PE array.  ALL matmuls run 2x column-tiled (tile_size
    (128,64)): tile (0,0) -> PSUM partitions 0-63, tile (0,64) ->
    64-127, each with its own moving fltr stream.  Measured 519 ns per
    kt-tile (2 concurrent 512-wide fp8 streams) vs 1034 serial -> PE
    ~67us per pass instead of ~110, making the kernel DMA-bound.
  - Every matmul in the kernel keeps tile_size (128,64) (no mode
    switches): K=64 matmuls (Y0 = x@w1, Y1 = relu@w1i1, w2-terms,
    final stack-mean) are zero-padded to K=128, with zeros placed in
    the operand that multiplies the junk rows.  The final stack-mean
    (out = 0.5*(relu_lo + relu_hi)) runs on the PE via a 0/1
    selection stationary instead of partition-shift DMA + DVE add.
  - The x@w2 bias term is accumulated LAST (stop) instead of first,
    so xm's DMA is off the critical path; the first fltr matmul
    carries start=True.
  - SBUF pinning: the pass-1 fltr blocks that pass-2 phase A (gather
    half 0, low core-blocks) consumes are kept resident (JPIN blocks
    per half-array stream); phase-A matmuls for those blocks are
    interleaved into pass-1 half-1's DMA-bound stream, filling PE
    idle slots.  Pass 2 re-reads only the rest.
  - Big fltr DMAs ride the sync-engine HWDGE ring; small/latency DMAs
    ride the scalar-engine ring; collectives keep the gpsimd queue.
    A dummy warm-up collective at t=0 absorbs the one-time rendezvous
    barrier (~60us) that would otherwise delay gather 0.
"""

import numpy as np
import ml_dtypes

import concourse.mybir as mybir
import concourse.tile as tile
from concourse import bacc
from concourse.bass_utils import run_bass_kernel_spmd

N = 16384            # nodes
F = 64               # input features
C = 32               # channels per stack
C2 = 2 * C           # fused channels (2 stacks)
NCORES = 8
R = N // NCORES      # fltr rows per core (2048)
P = 128              # partitions
NKT = N // P         # kt tiles per full pass (128)
HW_ = R // 2         # 1024 output rows per half-array
CW = 512             # output rows per chunk / PSUM accumulator slice
KB1 = 4              # kt tiles per fltr DMA block (512 KiB)
NBLK = NKT // KB1    # 32 blocks per half-array
JPIN = 6             # pin blocks {4j,4j+1: j<JPIN} of both half-arrays
FSCALE = 256.0       # power-of-2 fp8 pre-scale (folded into w1)

F32 = mybir.dt.float32
F32R = mybir.dt.float32r
BF16 = mybir.dt.bfloat16
F8 = mybir.dt.float8e3

_CACHE = {}


PHASES = []


def _mark(nc, label):
    PHASES.append((label, sum(1 for _ in nc.all_instructions())))


def _build():
    nc = bacc.Bacc(
        trn_type="TRN2", target_bir_lowering=False, debug=False,
        num_devices=NCORES,
    )
    # block-interleaved fltr: row ktb*128+p holds that partition's 4
    # kt-rows contiguously -> 4 KiB DMA descriptors (near-ceiling HBM rate)
    fltrT0_e = nc.dram_tensor("fltrt0", [NBLK * P, KB1 * HW_], F8,
                              kind="ExternalInput")
    fltrT1_e = nc.dram_tensor("fltrt1", [NBLK * P, KB1 * HW_], F8,
                              kind="ExternalInput")
    xt_e = nc.dram_tensor("xt", [P, N // 2], BF16, kind="ExternalInput")
    xtm_e = nc.dram_tensor("xtm", [P, R], BF16, kind="ExternalInput")
    w1i0_e = nc.dram_tensor("w1i0p", [P, P], BF16, kind="ExternalInput")
    w1i1_e = nc.dram_tensor("w1i1p", [P, P], BF16, kind="ExternalInput")
    w2i0_e = nc.dram_tensor("w2i0p", [P, C2], BF16, kind="ExternalInput")
    w2i1_e = nc.dram_tensor("w2i1p", [P, C2], BF16, kind="ExternalInput")
    bi0_e = nc.dram_tensor("bi0d", [P, 1], F32, kind="ExternalInput")
    bi1h_e = nc.dram_tensor("bi1hd", [P, 1], F32, kind="ExternalInput")
    ssum_e = nc.dram_tensor("ssum", [P, P], BF16, kind="ExternalInput")
    out_e = nc.dram_tensor("out", [C, R], F32, kind="ExternalOutput")

    RG = [list(range(NCORES))]
    fltr_halves = [fltrT0_e, fltrT1_e]

    with tile.TileContext(nc) as tc:
        with (
            tc.tile_pool(name="wpool", bufs=1) as wpool,
            tc.tile_pool(name="y0pool", bufs=1) as y0pool,
            tc.tile_pool(name="xtq", bufs=4) as xtqpool,
            tc.tile_pool(name="kpool", bufs=1) as kpool,
            tc.tile_pool(name="fpool", bufs=10) as fpool,
            tc.tile_pool(name="xbpool", bufs=2) as xbpool,
            tc.tile_pool(name="ylpool", bufs=2) as ylpool,
            tc.tile_pool(name="pacc", bufs=4, space="PSUM") as pacc,
            tc.tile_pool(name="psmall", bufs=2, space="PSUM") as psmall,
            tc.tile_pool(name="dram", bufs=8, space="DRAM") as dram,
        ):
            # ---- resident small tensors (scalar ring) + xt (sync ring,
            # ---- ahead of the fltr stream; 4 independent tiles)
            w1i0p = wpool.tile([P, P], BF16)
            nc.scalar.dma_start(w1i0p[:], w1i0_e[:])
            xts = []
            for q in range(4):
                xq = xtqpool.tile([P, N // 8], BF16, name=f"xt{q}",
                                  tag="xtq")
                nc.scalar.dma_start(xq[:],
                                    xt_e[:, q * (N // 8):(q + 1) * (N // 8)])
                xts.append(xq)

            # dummy warm-up collective: anchors the one-time rendezvous
            # barrier while all cores are still in startup.
            gwin = dram.tile([F, C2], BF16, name="gwin", tag="gwin")
            nc.scalar.dma_start(gwin[:], w1i0p[0:F, 0:C2])
            gwout = dram.tile([NCORES * F, C2], BF16, name="gwout",
                              tag="gwout", addr_space="Shared")
            nc.gpsimd.collective_compute(
                "AllGather", mybir.AluOpType.bypass,
                replica_groups=RG,
                ins=[gwin[:].opt()], outs=[gwout[:].opt()],
            )

            w1i1p = wpool.tile([P, P], BF16)
            nc.scalar.dma_start(w1i1p[:], w1i1_e[:])
            w2i0p = wpool.tile([P, C2], BF16)
            nc.scalar.dma_start(w2i0p[:], w2i0_e[:])
            w2i1p = wpool.tile([P, C2], BF16)
            nc.scalar.dma_start(w2i1p[:], w2i1_e[:])
            bi0d = wpool.tile([P, 1], F32)
            nc.scalar.dma_start(bi0d[:], bi0_e[:])
            bi1hd = wpool.tile([P, 1], F32)
            nc.scalar.dma_start(bi1hd[:], bi1h_e[:])
            ssum = wpool.tile([P, P], BF16)
            nc.scalar.dma_start(ssum[:], ssum_e[:])
            xm = wpool.tile([P, R], BF16)
            nc.scalar.dma_start(xm[:], xtm_e[:])

            y0 = y0pool.tile([P, NKT * C2], BF16, tag="y0")

            def y0_block(b):
                # twin-node: one N=128 matmul computes Y0 for a 64-node
                # lo-half group (xt rows 0-63 x w1i0p cols 0:64) AND its
                # hi-half twin (rows 64-127 x cols 64:128).  Block b
                # covers lo nodes [512b, 512b+512) = lo kt 4b..4b+4 and
                # their twins kt 64+4b..; psum placement makes the
                # evacuation one contiguous [128, 512] copy.
                ps = psmall.tile([P, 4, P], F32, name="ps0", tag="ps0")
                q4 = b // 4
                base = b * 512 - q4 * (N // 8)
                xq = xts[q4]
                for q in range(8):
                    col = base + q * C2
                    nc.tensor.matmul(
                        ps[(q % 2) * C2:(q % 2) * C2 + C2, q // 2, :],
                        xq[:, col:col + C2], w1i0p[:],
                        start=True, stop=True,
                        tile_position=(0, (q % 2) * C2),
                    )
                nc.vector.tensor_copy(
                    y0[:, b * CW:(b + 1) * CW],
                    ps[:].rearrange("p u c -> p (u c)"))

            # ---- pass-1 state
            p1acc = [None, None]
            kept = {}

            def p1_dma_block(h, ktb):
                if ktb % 4 < 2 and ktb // 4 < JPIN:
                    ft = kpool.tile([P, KB1, HW_], F8, name="ftk",
                                    tag="ftk", bufs=4 * JPIN)
                    kept[(h, ktb)] = ft
                else:
                    ft = fpool.tile([P, KB1, HW_], F8, name="ft", tag="ft")
                nc.sync.dma_start(
                    ft[:],
                    fltr_halves[h][ktb * P:(ktb + 1) * P, :]
                    .rearrange("p (b c) -> p b c", c=HW_),
                )
                return ft

            def y0sl(kt):
                if kt < NKT // 2:
                    return y0[:, kt * P:kt * P + C2]
                return y0[:, (kt - NKT // 2) * P + C2:(kt - NKT // 2) * P + P]

            def p1_mms(h, ktb, ft):
                acc = p1acc[h]
                for b in range(KB1):
                    kt = ktb * KB1 + b
                    first = kt == 0
                    yt = y0sl(kt)
                    nc.tensor.matmul(acc[0:C2, :], yt, ft[:, b, 0:CW],
                                     start=first, stop=False,
                                     tile_position=(0, 0))
                    nc.tensor.matmul(acc[C2:P, :], yt, ft[:, b, CW:HW_],
                                     start=first, stop=False,
                                     tile_position=(0, C2))

            def p1_epilogue(h):
                ctx_p = tc.high_priority(offset=600)
                ctx_p.__enter__()
                acc = p1acc[h]
                # x@w2 term, contracted over zero-padded K=128 (stop)
                nc.tensor.matmul(acc[0:C2, :], w2i0p[:],
                                 xm[:, 2 * h * CW:(2 * h + 1) * CW],
                                 start=False, stop=True,
                                 tile_position=(0, 0))
                nc.tensor.matmul(acc[C2:P, :], w2i0p[:],
                                 xm[:, (2 * h + 1) * CW:(2 * h + 2) * CW],
                                 start=False, stop=True,
                                 tile_position=(0, C2))
                xb1 = xbpool.tile([P, CW], BF16, name="xb1")
                nc.scalar.activation(
                    xb1[0:C2, :], acc[0:C2, :],
                    mybir.ActivationFunctionType.Relu,
                    bias=bi0d[0:C2, :], scale=1.0,
                )
                nc.scalar.activation(
                    xb1[C2:P, :], acc[C2:P, :],
                    mybir.ActivationFunctionType.Relu,
                    bias=bi0d[C2:P, :], scale=1.0,
                )
                psy = psmall.tile([P, 4, P], F32, name="psy", tag="ps0")
                for g in range(8):
                    nc.tensor.matmul(
                        psy[(g % 2) * C2:(g % 2) * C2 + C2, g // 2, :],
                        xb1[:, g * C2:(g + 1) * C2], w1i1p[:],
                        start=True, stop=True,
                        tile_position=(0, (g % 2) * C2),
                    )
                y1h = ylpool.tile([P, 8, C2], BF16, name="y1h")
                nc.vector.tensor_copy(y1h[:, 0:4, :], psy[:, :, 0:C2])
                nc.vector.tensor_copy(y1h[:, 4:8, :], psy[:, :, C2:P])
                # two sub-gathers per half (t 0..3 / 4..7) so pass-2 can
                # start on the first 64 KiB while the second is in flight
                # gin DMAs ride SWDGE: separate completion-sem accounting
                # (HWDGE's 8 lanes are shared with the fltr stream, which
                # adds ~15us of false ordering waits to the doorbell)
                for part in range(2):
                    gin = dram.tile([P, 4 * C2], BF16, name=f"gin{part}",
                                    tag=f"gin{part}", bufs=2)
                    nc.gpsimd.dma_start(
                        gin[:],
                        y1h[:, 4 * part:4 * part + 4, :]
                        .rearrange("p t ch -> p (t ch)"))
                    gout = dram.tile(
                        [NCORES * P, 4 * C2], BF16, name=f"gout{part}",
                        tag=f"gout{part}", addr_space="Shared", bufs=2,
                    )
                    nc.gpsimd.collective_compute(
                        "AllGather", mybir.AluOpType.bypass,
                        replica_groups=RG,
                        ins=[gin[:].opt()], outs=[gout[:].opt()],
                    )
                    gouts.append(gout)
                ctx_p.__exit__(None, None, None)

            gouts = []
            ygt = [None] * 4
            yg_issued = [False] * 2

            def issue_yg(hg):
                if yg_issued[hg]:
                    return
                with tc.high_priority(offset=600):
                    for part in range(2):
                        t = xtqpool.tile([P, N // 8], BF16,
                                         name=f"yg{hg}{part}", tag="xtq")
                        ygt[hg * 2 + part] = t
                        # SWDGE: rides the gpsimd queue, FIFO behind the
                        # gather it consumes; keeps the ACT ring free for
                        # the next epilogue's gin DMA
                        nc.gpsimd.dma_start(
                            t[:].rearrange("p (m c) -> p m c", c=4 * C2),
                            gouts[hg * 2 + part][:]
                            .rearrange("(m p) c -> p m c", p=P),
                        )
                yg_issued[hg] = True

            # ---- pass-2 state
            p2acc = [None, None]   # [chunks 0|1, chunks 2|3]

            def p2_init():
                p2acc[0] = pacc.tile([P, CW], F32, name="p2a", tag="acc")
                p2acc[1] = pacc.tile([P, CW], F32, name="p2b", tag="acc")
                for pair in range(2):
                    for t in range(2):
                        ck = 2 * pair + t
                        nc.tensor.matmul(
                            p2acc[pair][t * C2:(t + 1) * C2, :],
                            w2i1p[:], xm[:, ck * CW:(ck + 1) * CW],
                            start=True, stop=False,
                            tile_position=(0, t * C2),
                        )

            def p2_tile(hg, j, i, ft0, ft1, stop=False):
                b = i % 4
                yt = ygt[hg * 2 + i // 4][
                    :, (j * 4 + i % 4) * C2:(j * 4 + i % 4 + 1) * C2]
                nc.tensor.matmul(p2acc[0][0:C2, :], yt, ft0[:, b, 0:CW],
                                 start=False, stop=stop,
                                 tile_position=(0, 0))
                nc.tensor.matmul(p2acc[0][C2:P, :], yt, ft0[:, b, CW:HW_],
                                 start=False, stop=stop,
                                 tile_position=(0, C2))
                if ft1 is None:
                    return
                nc.tensor.matmul(p2acc[1][0:C2, :], yt, ft1[:, b, 0:CW],
                                 start=False, stop=stop,
                                 tile_position=(0, 0))
                nc.tensor.matmul(p2acc[1][C2:P, :], yt, ft1[:, b, CW:HW_],
                                 start=False, stop=stop,
                                 tile_position=(0, C2))

            def p2_stream_block(hg, j, blk_i):
                # DMA half-array 0/1 blocks for (j, i in [4*blk_i,+4))
                ktb = j * 4 + 2 * hg + blk_i
                fts = []
                for ha in range(2):
                    ft = kept.get((ha, ktb))
                    if ft is None:
                        ft = fpool.tile([P, KB1, HW_], F8, name="ft2",
                                        tag="ft")
                        nc.sync.dma_start(
                            ft[:],
                            fltr_halves[ha][ktb * P:(ktb + 1) * P, :]
                            .rearrange("p (b c) -> p b c", c=HW_),
                        )
                    fts.append(ft)
                return fts

            def p2_epilogue(pair):
                ctx_p = tc.high_priority(offset=600)
                ctx_p.__enter__()
                acc = p2acc[pair]
                xb2 = xbpool.tile([P, CW], BF16, name="xb2")
                for t in range(2):
                    nc.scalar.activation(
                        xb2[t * C2:(t + 1) * C2, :],
                        acc[t * C2:(t + 1) * C2, :],
                        mybir.ActivationFunctionType.Relu,
                        bias=bi1hd[t * C2:(t + 1) * C2, :], scale=0.5,
                    )
                pso = psmall.tile([P, CW], F32, name="pso", tag="ps0")
                for t in range(2):
                    nc.tensor.matmul(
                        pso[t * C2:(t + 1) * C2, :],
                        ssum[:, t * C2:(t + 1) * C2],
                        xb2[:],
                        start=True, stop=True,
                        tile_position=(0, t * C2),
                    )
                oT = xbpool.tile([P, CW], F32, name="oT")
                nc.vector.tensor_copy(oT[:], pso[:])
                for t in range(2):
                    ck = 2 * pair + t
                    nc.scalar.dma_start(
                        out_e[:, ck * CW:(ck + 1) * CW],
                        oT[t * C2:t * C2 + C, :],
                    )
                ctx_p.__exit__(None, None, None)

            # ================= emission =================
            # pass-1 half 0 (+ Y0 interleaved)
            _mark(nc, "p1h0_start")
            p1acc[0] = pacc.tile([P, CW], F32, name="p1a", tag="acc")
            for ktb in range(NBLK):
                if ktb < 16:
                    y0_block(ktb)
                ft = p1_dma_block(0, ktb)
                p1_mms(0, ktb, ft)
            _mark(nc, "p1h0_end")
            p1_epilogue(0)
            _mark(nc, "epi0_end")

            # pass-1 half 1; yg0 load + p2 acc init issued mid-half so
            # phase A can start the moment half-1's matmuls finish
            import os as _os
            ilv = _os.environ.get("ARMA_ILV") == "1"
            jilv = 3 if ilv else 0
            p1acc[1] = pacc.tile([P, CW], F32, name="p1b", tag="acc")
            for ktb in range(NBLK):
                ft = p1_dma_block(1, ktb)
                p1_mms(1, ktb, ft)
                if ktb == 8:
                    issue_yg(0)
                if ktb == 12:
                    p2_init()
                if ilv and ktb >= 16 and ktb % 4 == 0 and (ktb - 16) // 4 < 3:
                    j = (ktb - 16) // 4
                    for i in range(8):
                        ktb2 = j * 4 + i // 4
                        p2_tile(0, j, i, kept[(0, ktb2)], kept[(1, ktb2)])
            _mark(nc, "p1h1_end")
            p1_epilogue(1)
            _mark(nc, "epi1_end")
            issue_yg(1)

            # phase A part-major: all i<4 tiles (sub-gather a), then i>=4
            for blk_i in range(2):
                for j in range(jilv, JPIN):
                    ktb2 = j * 4 + blk_i
                    for i in range(4 * blk_i, 4 * blk_i + 4):
                        p2_tile(0, j, i, kept[(0, ktb2)], kept[(1, ktb2)])
                if blk_i == 1:
                    continue
                for j in range(JPIN, NCORES):
                    fts = p2_stream_block(0, j, 0)
                    for i in range(0, 4):
                        p2_tile(0, j, i, fts[0], fts[1])
            _mark(nc, "phA_pinned_end")
            for j in range(JPIN, NCORES):
                fts = p2_stream_block(0, j, 1)
                for i in range(4, 8):
                    p2_tile(0, j, i, fts[0], fts[1])
            _mark(nc, "phA_end")
            # phase B part-major; last block split for epilogue hiding
            for blk_i in range(2):
                for j in range(NCORES):
                    lastblk = j == NCORES - 1 and blk_i == 1
                    if lastblk:
                        continue
                    fts = p2_stream_block(1, j, blk_i)
                    for i in range(4 * blk_i, 4 * blk_i + 4):
                        p2_tile(1, j, i, fts[0], fts[1])
            # last block (j=7, part 1): chunks 0,1 finish + epilogue
            # while chunks 2,3 still accumulate, hiding the epilogue
            j = NCORES - 1
            fts = p2_stream_block(1, j, 1)
            for i in range(4, 8):
                b = i % 4
                yt = ygt[3][:, (j * 4 + i % 4) * C2:
                            (j * 4 + i % 4 + 1) * C2]
                nc.tensor.matmul(
                    p2acc[0][0:C2, :], yt, fts[0][:, b, 0:CW],
                    start=False, stop=(i == 7), tile_position=(0, 0))
                nc.tensor.matmul(
                    p2acc[0][C2:P, :], yt, fts[0][:, b, CW:HW_],
                    start=False, stop=(i == 7), tile_position=(0, C2))
            p2_epilogue(0)
            for i in range(4, 8):
                b = i % 4
                yt = ygt[3][:, (j * 4 + i % 4) * C2:
                            (j * 4 + i % 4 + 1) * C2]
                nc.tensor.matmul(
                    p2acc[1][0:C2, :], yt, fts[1][:, b, 0:CW],
                    start=False, stop=(i == 7), tile_position=(0, 0))
                nc.tensor.matmul(
                    p2acc[1][C2:P, :], yt, fts[1][:, b, CW:HW_],
                    start=False, stop=(i == 7), tile_position=(0, C2))
            p2_epilogue(1)

    nc.compile()
    return nc


def kernel(**inputs):
    x = np.ascontiguousarray(np.asarray(inputs["x"], dtype=np.float32))
    fltr = np.ascontiguousarray(np.asarray(inputs["fltr"], dtype=np.float32))

    def cat(a, b, axis=1):
        return np.ascontiguousarray(
            np.concatenate(
                [np.asarray(a, np.float32), np.asarray(b, np.float32)],
                axis=axis,
            )
        )

    f8 = ml_dtypes.float8_e3m4
    bf = ml_dtypes.bfloat16

    # fused conv kernels, descaled by 2^-8 (fp8 fold)
    w1i0f = (cat(inputs["k0i0_w1"], inputs["k1i0_w1"]) / FSCALE)  # [64,64]
    w1i1f = np.zeros((C2, C2), dtype=np.float32)
    w1i1f[0:C, 0:C] = np.asarray(inputs["k0i1_w1"], np.float32)
    w1i1f[C:C2, C:C2] = np.asarray(inputs["k1i1_w1"], np.float32)
    w1i1f = w1i1f / FSCALE

    def dpad(w):  # [[w,0],[0,w]] -> [128, 128]
        o = np.zeros((P, P), dtype=np.float32)
        o[0:C2, 0:C2] = w
        o[C2:P, C2:P] = w
        return o

    w1i0p = np.ascontiguousarray(dpad(w1i0f).astype(bf))
    w1i1p = np.ascontiguousarray(dpad(w1i1f).astype(bf))

    def zpad(w):  # [w; 0] -> [128, 64]
        o = np.zeros((P, C2), dtype=np.float32)
        o[0:C2, :] = w
        return np.ascontiguousarray(o)

    w2i0p = zpad(cat(inputs["k0i0_w2"], inputs["k1i0_w2"])).astype(bf)
    w2i1p = zpad(cat(inputs["k0i1_w2"], inputs["k1i1_w2"])).astype(bf)
    bi0d = np.ascontiguousarray(
        np.tile(cat(inputs["k0i0_b"], inputs["k1i0_b"], axis=0), 2)[:, None])
    bi1hd = np.ascontiguousarray(
        0.5 * np.tile(cat(inputs["k0i1_b"], inputs["k1i1_b"], axis=0),
                      2)[:, None])

    # stack-mean selection: out[m] = xb2[m] + xb2[m+32] for m < 32
    ssum = np.zeros((P, P), dtype=np.float32)
    for m in range(C):
        ssum[m, m] = ssum[m + C, m] = 1.0
        ssum[C2 + m, C2 + m] = ssum[C2 + C + m, C2 + m] = 1.0
    ssum = np.ascontiguousarray(ssum.astype(bf))

    xT = x.T.astype(bf)                      # [64, 16384]
    xt = np.ascontiguousarray(
        np.concatenate([xT[:, 0:N // 2], xT[:, N // 2:N]], axis=0))

    # fp8 E3M4 fltr at rest, transposed per core, scaled by 2^8
    fltrs = (fltr * np.float32(FSCALE)).astype(f8)

    if "nc" not in _CACHE:
        _CACHE["nc"] = _build()
    nc = _CACHE["nc"]

    in_maps = []
    for m in range(NCORES):
        rows = slice(m * R, (m + 1) * R)
        xtm = np.zeros((P, R), dtype=np.float32)
        xtm[0:F, :] = x[rows, :].T
        xtm = xtm.astype(bf)
        def blk(a):  # [N, HW_] -> [NBLK*P, KB1*HW_] block-interleaved
            return np.ascontiguousarray(
                a.reshape(NBLK, KB1, P, HW_).transpose(0, 2, 1, 3)
                .reshape(NBLK * P, KB1 * HW_))

        in_maps.append({
            "fltrt0": blk(np.ascontiguousarray(
                fltrs[m * R:m * R + HW_, :].T)),
            "fltrt1": blk(np.ascontiguousarray(
                fltrs[m * R + HW_:(m + 1) * R, :].T)),
            "xt": xt,
            "xtm": np.ascontiguousarray(xtm),
            "w1i0p": w1i0p, "w1i1p": w1i1p,
            "w2i0p": w2i0p, "w2i1p": w2i1p,
            "bi0d": bi0d, "bi1hd": bi1hd, "ssum": ssum,
        })

    import os
    import time
    trace = os.environ.get("ARMA_TRACE") == "1"
    last_exc = None
    for attempt in range(3):
        try:
            res = run_bass_kernel_spmd(
                nc, in_maps, core_ids=list(range(NCORES)), trace=trace,
            )
            break
        except Exception as e:  # transient NRT device errors: retry
            last_exc = e
            time.sleep(5.0)
    else:
        raise last_exc
    _CACHE["last_results"] = res
    out = np.concatenate(
        [np.asarray(res.results[m]["out"]).T for m in range(NCORES)], axis=0
    )
    return out
